# revision 1
# baseline (speedup 1.0000x reference)
"""ChildSum TreeLSTM cell for 8 Trainium2 NeuronCores — self-contained kernel.

Sharding: nodes and edges are partitioned by edge_dst owner across the 8
cores (25000 nodes each). Within a core, nodes are permuted into 98 blocks
of 256 destination nodes, balanced so ~90 blocks carry at most 256 edges
(2 edge chunks) and the last 8 blocks absorb heavy nodes (384 edges, 3
chunks). During input staging the host materializes the halo — h[src],
c[src] rows for every edge in block order (plus a feature-major copy of
h[src] for the forget-gate matmul) — so the device kernel is pure
streaming DMA + fp32r matmuls. Small weights (U_f, U_iou, W_iou) are
replicated on every core. Outputs come back feature-major and permuted;
the host inverts both.

Device pipeline, per group of G=2 blocks (512 destination nodes):
  per block:  f = sigmoid(h_child @ U_f_w.T [+ U_f_b])   (per-edge, PE)
              h_tildT += h_child.T @ S ; c_aggT += (f*c_child).T @ S
                (S = one-hot dst selection built on DVE via iota/is_equal)
  per group:  iouT[fo] = sum_fi W.T[fi,fo] @ xT + U.T[fi,fo] @ h_tildT
              i,o,u = sigmoid/sigmoid/tanh(iouT + b_iou)  (ACT, bias/partition)
              c_newT = i*u + c_aggT ; h_newT = o * tanh(c_newT)
"""
import os
import sys

for _p in ("/opt/trn_rl_repo",):
    if _p not in sys.path:
        sys.path.insert(0, _p)

import heapq

import numpy as np

import concourse.bass as bass
import concourse.bacc as bacc
import concourse.mybir as mybir
import concourse.tile as tile
from concourse.bass_utils import run_bass_kernel_spmd

f32 = mybir.dt.float32
f32r = mybir.dt.float32r

N_CORES = 8
BLK = 256   # destination nodes per block
G = 2       # blocks per group (iou phase fusion)
H = 256
X = 256
OVER = 8    # trailing blocks with extra edge capacity

LAST_EXEC_TIME_NS = None
_PROGRAM_CACHE = {}


def _build_program(nblk, loc, ec_list, fbias_zero):
    FO = 3 * H // 128  # 6
    KH = H // 128      # 2
    GN = G * BLK       # nodes per group
    assert nblk % G == 0
    ecs = np.asarray(ec_list)
    # hslab: per block [128, ec*H (h_child, chunk-major) + KH*ec*128 (h_childT)]
    hcols = np.concatenate([[0], np.cumsum(ecs * H + KH * ecs * 128)])
    # cslab: per block [128, ec*H (c_child) + ec (dst_rel)]
    ccols = np.concatenate([[0], np.cumsum(ecs * H + ecs)])

    nc = bacc.Bacc(None, target_bir_lowering=False, debug=False)

    hsl_d = nc.declare_dram_parameter("hslab", [128, int(hcols[-1])], f32r,
                                      isOutput=False)
    csl_d = nc.declare_dram_parameter("cslab", [128, int(ccols[-1])], f32,
                                      isOutput=False)
    xT_d = nc.declare_dram_parameter("xT", [X, loc], f32r, isOutput=False)
    ufwT_d = nc.declare_dram_parameter("ufwT", [X, H], f32r, isOutput=False)
    wiouT_d = nc.declare_dram_parameter("wiouT", [X, 3 * H], f32r, isOutput=False)
    uiouT_d = nc.declare_dram_parameter("uiouT", [H, 3 * H], f32r, isOutput=False)
    bcol_d = nc.declare_dram_parameter("bcol", [128, FO], f32, isOutput=False)
    iota_d = nc.declare_dram_parameter("iota", [128, BLK], f32, isOutput=False)
    if not fbias_zero:
        ufb_d = nc.declare_dram_parameter("ufb", [1, H], f32r, isOutput=False)

    houtT_d = nc.declare_dram_parameter("houtT", [H, loc], f32, isOutput=True)
    coutT_d = nc.declare_dram_parameter("coutT", [H, loc], f32, isOutput=True)

    SIG = mybir.ActivationFunctionType.Sigmoid
    TANH = mybir.ActivationFunctionType.Tanh

    with tile.TileContext(nc) as tc:
        with (
            tc.tile_pool(name="const", bufs=1) as cpool,
            tc.tile_pool(name="io", bufs=4) as iop,
            tc.tile_pool(name="work", bufs=3) as wp,
            tc.tile_pool(name="grp", bufs=2) as gp,
            tc.tile_pool(name="ps", bufs=1, space="PSUM") as psp,
            tc.tile_pool(name="ps_acc", bufs=1, space="PSUM") as pacc,
        ):
            iota_t = cpool.tile([128, BLK], f32)
            nc.sync.dma_start(out=iota_t[:], in_=iota_d[:])
            bcol_t = cpool.tile([128, FO], f32)
            nc.sync.dma_start(out=bcol_t[:], in_=bcol_d[:])
            ufw_t = []
            for fi in range(KH):
                t = cpool.tile([128, H], f32r, tag=f"ufw{fi}", name=f"ufw{fi}")
                nc.sync.dma_start(out=t[:], in_=ufwT_d[fi * 128:(fi + 1) * 128, :])
                ufw_t.append(t)
            wiou_t = [[None] * FO for _ in range(KH)]
            uiou_t = [[None] * FO for _ in range(KH)]
            for fi in range(KH):
                for fo in range(FO):
                    t = cpool.tile([128, 128], f32r, tag=f"wiou{fi}_{fo}",
                                   name=f"wiou{fi}_{fo}")
                    nc.sync.dma_start(
                        out=t[:], in_=wiouT_d[fi * 128:(fi + 1) * 128,
                                              fo * 128:(fo + 1) * 128])
                    wiou_t[fi][fo] = t
                    t = cpool.tile([128, 128], f32r, tag=f"uiou{fi}_{fo}",
                                   name=f"uiou{fi}_{fo}")
                    nc.sync.dma_start(
                        out=t[:], in_=uiouT_d[fi * 128:(fi + 1) * 128,
                                              fo * 128:(fo + 1) * 128])
                    uiou_t[fi][fo] = t
            if not fbias_zero:
                ones_t = cpool.tile([1, 128], f32r)
                nc.vector.memset(ones_t[:].bitcast(f32), 1.0)
                ufb_t = cpool.tile([1, H], f32r)
                nc.sync.dma_start(out=ufb_t[:], in_=ufb_d[:])

            max_ec = max(ec_list)
            for g in range(nblk // G):
                xtg = gp.tile([128, KH, GN], f32r, tag="xtg")
                nc.sync.dma_start(
                    out=xtg[:],
                    in_=xT_d[:, g * GN:(g + 1) * GN].rearrange(
                        "(f p) c -> p f c", p=128))
                htg = [gp.tile([128, GN], f32r, tag=f"htg{fi}", name=f"htg{fi}")
                       for fi in range(KH)]
                cag = [gp.tile([128, GN], f32, tag=f"cag{fi}", name=f"cag{fi}")
                       for fi in range(KH)]

                for jj in range(G):
                    j = g * G + jj
                    ec = ec_list[j]
                    ne = ec * 128
                    h0 = int(hcols[j])
                    c0 = int(ccols[j])
                    hsl_t = iop.tile([128, max_ec * H * 2], f32r, tag="hsl")
                    nc.sync.dma_start(
                        out=hsl_t[:, :ec * H + KH * ne],
                        in_=hsl_d[:, h0:h0 + ec * H + KH * ne])
                    csl_t = iop.tile([128, max_ec * (H + 1)], f32, tag="csl")
                    nc.sync.dma_start(
                        out=csl_t[:, :ec * H + ec],
                        in_=csl_d[:, c0:c0 + ec * H + ec])

                    htild_ps = [pacc.tile([128, BLK], f32, tag=f"htild{fi}",
                                          name=f"htild_ps{fi}")
                                for fi in range(KH)]
                    cagg_ps = [pacc.tile([128, BLK], f32, tag=f"cagg{fi}",
                                         name=f"cagg_ps{fi}")
                               for fi in range(KH)]

                    hT0 = ec * H  # h_childT offset within hslab block
                    for ci in range(ec):
                        hch_c = hsl_t[:, ci * H:(ci + 1) * H]
                        cch_c = csl_t[:, ci * H:(ci + 1) * H]
                        s_t = wp.tile([128, BLK], f32r, tag="S")
                        nc.vector.tensor_scalar(
                            out=s_t[:], in0=iota_t[:],
                            scalar1=csl_t[:, ec * H + ci:ec * H + ci + 1],
                            scalar2=None,
                            op0=mybir.AluOpType.is_equal)
                        f_ps = psp.tile([128, H], f32, tag="f", bufs=2)
                        for fi in range(KH):
                            nc.tensor.matmul(
                                out=f_ps[:],
                                lhsT=hsl_t[:, hT0 + fi * ne + ci * 128:
                                           hT0 + fi * ne + ci * 128 + 128],
                                rhs=ufw_t[fi][:],
                                start=(fi == 0),
                                stop=(fi == KH - 1 and fbias_zero))
                        if not fbias_zero:
                            nc.tensor.matmul(out=f_ps[:], lhsT=ones_t[:],
                                             rhs=ufb_t[:],
                                             start=False, stop=True)
                        f_sb = wp.tile([128, H], f32, tag="fsb")
                        nc.scalar.activation(out=f_sb[:], in_=f_ps[:], func=SIG)
                        fc_t = wp.tile([128, H], f32r, tag="fc")
                        nc.vector.tensor_tensor(out=fc_t[:], in0=f_sb[:],
                                                in1=cch_c,
                                                op=mybir.AluOpType.mult)
                        for fi in range(KH):
                            nc.tensor.matmul(
                                out=htild_ps[fi][:],
                                lhsT=hch_c[:, fi * 128:(fi + 1) * 128],
                                rhs=s_t[:],
                                start=(ci == 0), stop=(ci == ec - 1))
                            nc.tensor.matmul(
                                out=cagg_ps[fi][:],
                                lhsT=fc_t[:, fi * 128:(fi + 1) * 128],
                                rhs=s_t[:],
                                start=(ci == 0), stop=(ci == ec - 1))

                    for fi in range(KH):
                        nc.vector.tensor_copy(
                            out=htg[fi][:, jj * BLK:(jj + 1) * BLK],
                            in_=htild_ps[fi][:])
                        nc.vector.tensor_copy(
                            out=cag[fi][:, jj * BLK:(jj + 1) * BLK],
                            in_=cagg_ps[fi][:])

                # ---- iou + apply for the whole group ----
                sb_act = []
                for fo in range(FO):
                    iou_ps = psp.tile([128, GN], f32, tag="iou", bufs=2)
                    first = True
                    for fi in range(KH):
                        nc.tensor.matmul(out=iou_ps[:], lhsT=wiou_t[fi][fo][:],
                                         rhs=xtg[:, fi, :], start=first,
                                         stop=False)
                        first = False
                    for fi in range(KH):
                        nc.tensor.matmul(out=iou_ps[:], lhsT=uiou_t[fi][fo][:],
                                         rhs=htg[fi][:], start=False,
                                         stop=(fi == KH - 1))
                    sb = wp.tile([128, GN], f32, tag=f"act{fo}", name=f"act{fo}")
                    nc.scalar.activation(out=sb[:], in_=iou_ps[:],
                                         func=(SIG if fo < 2 * KH else TANH),
                                         bias=bcol_t[:, fo:fo + 1])
                    sb_act.append(sb)

                for fi in range(KH):
                    i_sb, o_sb, u_sb = (sb_act[fi], sb_act[KH + fi],
                                        sb_act[2 * KH + fi])
                    iu = wp.tile([128, GN], f32, tag="iu")
                    nc.gpsimd.tensor_tensor(out=iu[:], in0=i_sb[:], in1=u_sb[:],
                                            op=mybir.AluOpType.mult)
                    cn = gp.tile([128, GN], f32, tag="cn")
                    nc.vector.tensor_tensor(out=cn[:], in0=iu[:],
                                            in1=cag[fi][:],
                                            op=mybir.AluOpType.add)
                    nc.scalar.dma_start(
                        out=coutT_d[fi * 128:(fi + 1) * 128,
                                    g * GN:(g + 1) * GN], in_=cn[:])
                    th = wp.tile([128, GN], f32, tag="th")
                    nc.scalar.activation(out=th[:], in_=cn[:], func=TANH)
                    hn = gp.tile([128, GN], f32, tag="hn")
                    nc.gpsimd.tensor_tensor(out=hn[:], in0=o_sb[:], in1=th[:],
                                            op=mybir.AluOpType.mult)
                    nc.scalar.dma_start(
                        out=houtT_d[fi * 128:(fi + 1) * 128,
                                    g * GN:(g + 1) * GN], in_=hn[:])

    nc.compile()
    return nc


def _pack_blocks(deg, nblk, caps_e):
    """Balanced bin packing: place nodes (desc degree) into blocks of 256
    node slots, respecting per-block edge capacities where possible."""
    npc = deg.shape[0]
    order = np.argsort(-deg, kind="stable")
    rem_e = caps_e.astype(np.int64).copy()
    rem_n = np.full(nblk, BLK, np.int64)
    heap = [(-rem_e[b], b) for b in range(nblk)]
    heapq.heapify(heap)
    assign = np.empty(npc, np.int64)
    for nd in order:
        d = int(deg[nd])
        tmp = []
        placed = False
        while heap:
            negre, b = heapq.heappop(heap)
            if rem_n[b] == 0:
                continue
            if d <= rem_e[b] or d == 0:
                rem_e[b] -= d
                rem_n[b] -= 1
                assign[nd] = b
                placed = True
                if rem_n[b] > 0:
                    heapq.heappush(heap, (-rem_e[b], b))
                break
            tmp.append((negre, b))
        for t in tmp:
            heapq.heappush(heap, t)
        if not placed:
            b = int(np.argmax(np.where(rem_n > 0, rem_e, -(1 << 60))))
            rem_e[b] -= d
            rem_n[b] -= 1
            assign[nd] = b
    blk_sorted = np.argsort(assign[order], kind="stable")
    perm = order[blk_sorted]  # nodes in block-major placement order
    pos = np.empty(npc, np.int64)
    used = np.bincount(assign, minlength=nblk)
    starts = np.concatenate([[0], np.cumsum(used)[:-1]])
    pos[perm] = np.arange(npc) - np.repeat(starts, used)
    return assign, pos, perm, used


def _prep_inputs(x, h, c, W_iou, U_iou, b_iou, U_f_w, U_f_b,
                 edge_src, edge_dst):
    n = x.shape[0]
    assert n % N_CORES == 0
    npc = n // N_CORES
    nblk = -(-npc // BLK)
    nblk = -(-nblk // G) * G  # group-aligned
    loc = nblk * BLK

    x = np.asarray(x, np.float32)
    h = np.asarray(h, np.float32)
    c = np.asarray(c, np.float32)
    edge_src = np.asarray(edge_src, np.int64)
    edge_dst = np.asarray(edge_dst, np.int64)

    n_over = min(OVER, nblk) if nblk > OVER else 0
    caps_e = np.array([BLK] * (nblk - n_over) + [BLK + 128] * n_over, np.int64)

    owner = edge_dst // npc
    cores = []
    ec_arr = np.zeros((N_CORES, nblk), np.int64)
    for k in range(N_CORES):
        m = owner == k
        ldst = edge_dst[m] - k * npc
        src = edge_src[m]
        deg = np.bincount(ldst, minlength=npc)
        assign, pos, perm, used = _pack_blocks(deg, nblk, caps_e)
        blk_id = assign[ldst]
        dstrel = pos[ldst]
        cnt = np.bincount(blk_id, minlength=nblk)
        ec_arr[k] = -(-np.maximum(cnt, 1) // 128)
        cores.append((src, blk_id, dstrel, cnt, perm, used))
    ec_list = tuple(int(v) for v in ec_arr.max(axis=0))
    ecs = np.asarray(ec_list)
    eoff = np.concatenate([[0], np.cumsum(ecs * 128)])
    tot_e = int(eoff[-1])
    hcols = np.concatenate([[0], np.cumsum(ecs * H + 2 * ecs * 128)])
    ccols = np.concatenate([[0], np.cumsum(ecs * H + ecs)])

    ufwT = np.ascontiguousarray(np.asarray(U_f_w, np.float32).T)
    wiouT = np.ascontiguousarray(np.asarray(W_iou, np.float32).T)
    uiouT = np.ascontiguousarray(np.asarray(U_iou, np.float32).T)
    b_iou_f = np.asarray(b_iou, np.float32).reshape(-1)
    bcol = np.ascontiguousarray(b_iou_f.reshape(3 * H // 128, 128).T)
    iota = np.broadcast_to(np.arange(BLK, dtype=np.float32), (128, BLK)).copy()
    U_f_b_f = np.asarray(U_f_b, np.float32).reshape(-1)
    fbias_zero = not U_f_b_f.any()

    in_maps = []
    perms = []
    for k in range(N_CORES):
        src, blk_id, dstrel, cnt, perm, used = cores[k]
        start = np.zeros(nblk, np.int64)
        np.cumsum(cnt[:-1], out=start[1:])
        eorder = np.argsort(blk_id, kind="stable")
        slot_in_blk = np.arange(blk_id.size) - start[blk_id[eorder]]
        flat_pos = eoff[blk_id[eorder]] + slot_in_blk
        hch = np.zeros((tot_e, H), np.float32)
        cch = np.zeros((tot_e, H), np.float32)
        hch[flat_pos] = h[src[eorder]]
        cch[flat_pos] = c[src[eorder]]
        flat_dst = np.full(tot_e, -1.0, np.float32)
        flat_dst[flat_pos] = dstrel[eorder].astype(np.float32)

        hslab = np.empty((128, int(hcols[-1])), np.float32)
        cslab = np.empty((128, int(ccols[-1])), np.float32)
        for j in range(nblk):
            ec = int(ecs[j])
            ne = ec * 128
            e0, e1 = int(eoff[j]), int(eoff[j + 1])
            hb = hch[e0:e1]                       # [ne, H]
            # h_child chunk-major: [p, ci, h]
            p1 = hb.reshape(ec, 128, H).transpose(1, 0, 2).reshape(128, ec * H)
            # h_childT: [p(feat within chunk), fi, e]
            p2 = hb.T.reshape(2, 128, ne).transpose(1, 0, 2).reshape(128, 2 * ne)
            hslab[:, int(hcols[j]):int(hcols[j]) + ec * H] = p1
            hslab[:, int(hcols[j]) + ec * H:int(hcols[j + 1])] = p2
            cb = cch[e0:e1]
            q1 = cb.reshape(ec, 128, H).transpose(1, 0, 2).reshape(128, ec * H)
            cslab[:, int(ccols[j]):int(ccols[j]) + ec * H] = q1
            cslab[:, int(ccols[j]) + ec * H:int(ccols[j + 1])] = \
                flat_dst[e0:e1].reshape(ec, 128).T

        xT = np.zeros((X, loc), np.float32)
        gperm = perm + k * npc
        cols = np.concatenate([
            np.arange(s, s + u) for s, u in zip(range(0, loc, BLK), used)])
        xT[:, cols] = x[gperm].T
        im = {
            "hslab": hslab, "cslab": cslab, "xT": xT,
            "ufwT": ufwT, "wiouT": wiouT, "uiouT": uiouT,
            "bcol": bcol, "iota": iota,
        }
        if not fbias_zero:
            im["ufb"] = U_f_b_f.reshape(1, H)
        in_maps.append(im)
        perms.append((gperm, cols))

    meta = dict(n=n, npc=npc, nblk=nblk, loc=loc, ec_list=ec_list,
                fbias_zero=fbias_zero, perms=perms)
    return in_maps, meta


def kernel(x, h, c, W_iou, U_iou, b_iou, U_f_w, U_f_b, edge_src, edge_dst,
           _trace=False):
    global LAST_EXEC_TIME_NS
    in_maps, meta = _prep_inputs(x, h, c, W_iou, U_iou, b_iou, U_f_w, U_f_b,
                                 edge_src, edge_dst)
    key = (meta["nblk"], meta["loc"], meta["ec_list"], meta["fbias_zero"])
    nc = _PROGRAM_CACHE.get(key)
    if nc is None:
        nc = _build_program(meta["nblk"], meta["loc"], meta["ec_list"],
                            meta["fbias_zero"])
        _PROGRAM_CACHE[key] = nc
    if not _trace:
        os.environ.setdefault("BASS_NEVER_TRACE", "1")
    res = run_bass_kernel_spmd(nc, in_maps, list(range(N_CORES)),
                               trace=_trace, trace_cores=[0] if _trace else None)
    if _trace:
        LAST_EXEC_TIME_NS = res.exec_time_ns

    n = meta["n"]
    h_new = np.empty((n, H), np.float32)
    c_new = np.empty((n, H), np.float32)
    for k in range(N_CORES):
        gperm, cols = meta["perms"][k]
        h_new[gperm] = res.results[k]["houtT"][:, cols].T
        c_new[gperm] = res.results[k]["coutT"][:, cols].T
    return h_new, c_new



# revision 10
# speedup vs baseline: 1.1439x; 1.1439x over previous
"""ChildSum TreeLSTM cell for 8 Trainium2 NeuronCores — self-contained kernel.

Sharding: nodes and edges partitioned by edge_dst owner across 8 cores
(25000 nodes each). Within a core, nodes are permuted into 200 blocks of
128 destination nodes. Nodes with zero in-degree (~37%, Poisson degree)
are segregated into trailing "zero" blocks whose groups skip the U-matmul
and aggregation entirely (iou = W x only, c_new = i*u). Dense blocks are
bin-packed so block edge counts land near 128/256 (1-2 chunks of 128
edges). The host stages the halo — h[src]/c[src] rows per edge in block
order, h[src] both edge-major and feature-major — in bf16, so the device
kernel is streaming DMA + bf16 matmuls (PSUM accumulates in f32).

Device pipeline per group of 8 blocks (1024 destination nodes):
  per chunk pair: f = sigmoid(h_childT.T @ U_f)        (PE + ACT)
                  fc = f * c_child                      (DVE)
  per chunk:      S = one-hot(dst) via iota/is_equal    (DVE)
                  h_tildT += h_child.T @ S              (PE, PSUM)
                  c_aggT  += fc.T @ S                   (PE, PSUM)
  per subround (4 blocks): copy PSUM -> bf16 SBUF       (DVE)
  iou halves:     iouT = W.T@xT [+ U.T@h_tildT]         (PE)
                  i,o = sigmoid, u = tanh               (ACT)
  apply:          iu = i*u (GpSimd); cn = iu + c_agg (DVE)
                  th = tanh(cn) (ACT); hn = o*th (GpSimd)
Outputs return feature-major and permuted; the host inverts both.
"""
import os
import sys

for _p in ("/opt/trn_rl_repo",):
    if _p not in sys.path:
        sys.path.insert(0, _p)

import heapq

import numpy as np
import ml_dtypes

import concourse.bass as bass
import concourse.bacc as bacc
import concourse.mybir as mybir
import concourse.tile as tile
from concourse.bass_utils import run_bass_kernel_spmd

f32 = mybir.dt.float32
bf16 = mybir.dt.bfloat16
BF = ml_dtypes.bfloat16

N_CORES = 8
BLK = 128    # destination nodes per block
G = 8        # blocks per group
GN = G * BLK # nodes per group
SUB = 4      # blocks per scatter subround
H = 256
X = 256
KH = H // 128   # 2
FO = 3 * H // 128  # 6

LAST_EXEC_TIME_NS = None
_PROGRAM_CACHE = {}


def _group_meta(ec_list, D8):
    """Per dense group: chunk table + slab offsets (shared across cores)."""
    ecs = list(ec_list)
    groups = []
    hbase = 0
    cbase = 0
    for g in range(D8 // G):
        blocks = ecs[g * G:(g + 1) * G]
        nch = sum(blocks)
        chunks = []  # (gchunk, block_local, cib, ec_of_block)
        gc = 0
        for bl, ec in enumerate(blocks):
            for c in range(ec):
                chunks.append((gc, bl, c, ec))
                gc += 1
        groups.append(dict(nch=nch, chunks=chunks, hbase=hbase, cbase=cbase,
                           dbase=sum(ecs[:g * G])))
        hbase += nch * 512
        cbase += nch * 256
    return groups, hbase, cbase


def _build_program(nblk, D8, loc, ec_list, fbias_zero):
    groups, htot, ctot = _group_meta(ec_list, D8)
    ngroups = nblk // G
    dense_groups = D8 // G
    max_h = max(g["nch"] for g in groups) * 512
    max_c = max(g["nch"] for g in groups) * 256
    max_n = max(g["nch"] for g in groups)
    tot_ch = sum(ec_list)

    nc = bacc.Bacc(None, target_bir_lowering=False, debug=False)

    hsl_d = nc.declare_dram_parameter("hslab", [128, htot], bf16, isOutput=False)
    csl_d = nc.declare_dram_parameter("cslab", [128, ctot], bf16, isOutput=False)
    drel_d = nc.declare_dram_parameter("drel", [128, tot_ch], f32, isOutput=False)
    xT_d = nc.declare_dram_parameter("xT", [X, loc], bf16, isOutput=False)
    ufwT_d = nc.declare_dram_parameter("ufwT", [X, H], bf16, isOutput=False)
    wiouT_d = nc.declare_dram_parameter("wiouT", [X, 3 * H], bf16, isOutput=False)
    uiouT_d = nc.declare_dram_parameter("uiouT", [H, 3 * H], bf16, isOutput=False)
    bcol_d = nc.declare_dram_parameter("bcol", [128, FO], f32, isOutput=False)
    iota_d = nc.declare_dram_parameter("iota", [128, BLK], bf16, isOutput=False)
    if not fbias_zero:
        ufb_d = nc.declare_dram_parameter("ufb", [1, H], bf16, isOutput=False)
        ones_d = nc.declare_dram_parameter("ones", [1, 128], bf16, isOutput=False)

    houtT_d = nc.declare_dram_parameter("houtT", [H, loc], bf16, isOutput=True)
    coutT_d = nc.declare_dram_parameter("coutT", [H, loc], bf16, isOutput=True)

    SIG = mybir.ActivationFunctionType.Sigmoid
    TANH = mybir.ActivationFunctionType.Tanh
    MUL = mybir.AluOpType.mult
    ADD = mybir.AluOpType.add

    with tile.TileContext(nc) as tc:
        with (
            tc.tile_pool(name="const", bufs=1) as cpool,
            tc.tile_pool(name="io", bufs=2) as iop,
            tc.tile_pool(name="work", bufs=3) as wp,
            tc.tile_pool(name="grp", bufs=2) as gp,
            tc.tile_pool(name="ps_f", bufs=2, space="PSUM") as psp,
            tc.tile_pool(name="ps_iou", bufs=2, space="PSUM") as psi,
            tc.tile_pool(name="ps_acc", bufs=1, space="PSUM") as pacc,
        ):
            iota_t = cpool.tile([128, BLK], bf16)
            nc.sync.dma_start(out=iota_t[:], in_=iota_d[:])
            bcol_t = cpool.tile([128, FO], f32)
            nc.sync.dma_start(out=bcol_t[:], in_=bcol_d[:])
            ufw_t = []
            for fi in range(KH):
                t = cpool.tile([128, H], bf16, tag=f"ufw{fi}", name=f"ufw{fi}")
                nc.sync.dma_start(out=t[:], in_=ufwT_d[fi * 128:(fi + 1) * 128, :])
                ufw_t.append(t)
            wiou_t = [[None] * FO for _ in range(KH)]
            uiou_t = [[None] * FO for _ in range(KH)]
            for fi in range(KH):
                for fo in range(FO):
                    t = cpool.tile([128, 128], bf16, tag=f"wiou{fi}_{fo}",
                                   name=f"wiou{fi}_{fo}")
                    nc.sync.dma_start(
                        out=t[:], in_=wiouT_d[fi * 128:(fi + 1) * 128,
                                              fo * 128:(fo + 1) * 128])
                    wiou_t[fi][fo] = t
                    t = cpool.tile([128, 128], bf16, tag=f"uiou{fi}_{fo}",
                                   name=f"uiou{fi}_{fo}")
                    nc.sync.dma_start(
                        out=t[:], in_=uiouT_d[fi * 128:(fi + 1) * 128,
                                              fo * 128:(fo + 1) * 128])
                    uiou_t[fi][fo] = t
            if not fbias_zero:
                ones_t = cpool.tile([1, 128], bf16)
                nc.sync.dma_start(out=ones_t[:], in_=ones_d[:])
                ufb_t = cpool.tile([1, H], bf16)
                nc.sync.dma_start(out=ufb_t[:], in_=ufb_d[:])

            for g in range(ngroups):
                dense = g < dense_groups
                xtg = gp.tile([128, KH, GN], bf16, tag="xtg")
                nc.sync.dma_start(
                    out=xtg[:],
                    in_=xT_d[:, g * GN:(g + 1) * GN].rearrange(
                        "(f p) c -> p f c", p=128))

                if dense:
                    gm = groups[g]
                    nch = gm["nch"]
                    hsl_t = iop.tile([128, max_h], bf16, tag="hsl")
                    nc.sync.dma_start(
                        out=hsl_t[:, :nch * 512],
                        in_=hsl_d[:, gm["hbase"]:gm["hbase"] + nch * 512])
                    csl_t = iop.tile([128, max_c], bf16, tag="csl")
                    nc.sync.dma_start(
                        out=csl_t[:, :nch * 256],
                        in_=csl_d[:, gm["cbase"]:gm["cbase"] + nch * 256])
                    drel_t = iop.tile([128, max_n], f32, tag="drel")
                    nc.sync.dma_start(
                        out=drel_t[:, :nch],
                        in_=drel_d[:, gm["dbase"]:gm["dbase"] + nch])
                    htg = gp.tile([128, KH, GN], bf16, tag="htg")
                    cag = gp.tile([128, KH, GN], bf16, tag="cag")
                    HT0 = nch * 256  # h_childT section offset

                    for sub in range(2):
                        L = [ch for ch in gm["chunks"] if ch[1] // SUB == sub]
                        ht_ps = [pacc.tile([128, SUB * BLK], f32, tag=f"ht{fi}",
                                           name=f"ht_ps{fi}")
                                 for fi in range(KH)]
                        ca_ps = [pacc.tile([128, SUB * BLK], f32, tag=f"ca{fi}",
                                           name=f"ca_ps{fi}")
                                 for fi in range(KH)]
                        for p0 in range(0, len(L), 2):
                            pair = L[p0:p0 + 2]
                            w = 256 * len(pair)
                            gc0 = pair[0][0]
                            f_ps = psp.tile([128, 512], f32, tag="f")
                            for q, (gc, bl, cib, ec) in enumerate(pair):
                                for fi in range(KH):
                                    nc.tensor.matmul(
                                        out=f_ps[:, q * 256:(q + 1) * 256],
                                        lhsT=hsl_t[:, HT0 + gc * 256 + fi * 128:
                                                   HT0 + gc * 256 + fi * 128 + 128],
                                        rhs=ufw_t[fi][:],
                                        start=(fi == 0),
                                        stop=(fi == KH - 1 and fbias_zero))
                                if not fbias_zero:
                                    nc.tensor.matmul(
                                        out=f_ps[:, q * 256:(q + 1) * 256],
                                        lhsT=ones_t[:], rhs=ufb_t[:],
                                        start=False, stop=True)
                            f_sb = wp.tile([128, 512], bf16, tag="fsb")
                            nc.scalar.activation(out=f_sb[:, :w],
                                                 in_=f_ps[:, :w], func=SIG)
                            fc = wp.tile([128, 512], bf16, tag="fc")
                            nc.vector.tensor_tensor(
                                out=fc[:, :w], in0=f_sb[:, :w],
                                in1=csl_t[:, gc0 * 256:gc0 * 256 + w], op=MUL)
                            for q, (gc, bl, cib, ec) in enumerate(pair):
                                s_t = wp.tile([128, BLK], bf16, tag="S")
                                nc.vector.tensor_scalar(
                                    out=s_t[:], in0=iota_t[:],
                                    scalar1=drel_t[:, gc:gc + 1],
                                    scalar2=None,
                                    op0=mybir.AluOpType.is_equal)
                                bl4 = bl % SUB
                                first = (cib == 0)
                                last = (cib == ec - 1)
                                for fi in range(KH):
                                    nc.tensor.matmul(
                                        out=ht_ps[fi][:, bl4 * BLK:(bl4 + 1) * BLK],
                                        lhsT=hsl_t[:, gc * 256 + fi * 128:
                                                   gc * 256 + fi * 128 + 128],
                                        rhs=s_t[:], start=first, stop=last)
                                    nc.tensor.matmul(
                                        out=ca_ps[fi][:, bl4 * BLK:(bl4 + 1) * BLK],
                                        lhsT=fc[:, q * 256 + fi * 128:
                                                q * 256 + fi * 128 + 128],
                                        rhs=s_t[:], start=first, stop=last)
                        SW = SUB * BLK
                        for fi in range(KH):
                            nc.vector.tensor_copy(
                                out=htg[:, fi, sub * SW:(sub + 1) * SW],
                                in_=ht_ps[fi][:])
                            nc.vector.tensor_copy(
                                out=cag[:, fi, sub * SW:(sub + 1) * SW],
                                in_=ca_ps[fi][:])

                # ---- iou for the whole group, in 512-node halves ----
                sb_act = [wp.tile([128, GN], bf16, tag=f"act{fo}",
                                  name=f"act{fo}") for fo in range(FO)]
                for fo in range(FO):
                    for hf in range(2):
                        iou_ps = psi.tile([128, 512], f32, tag="iou")
                        for fi in range(KH):
                            nc.tensor.matmul(
                                out=iou_ps[:], lhsT=wiou_t[fi][fo][:],
                                rhs=xtg[:, fi, hf * 512:(hf + 1) * 512],
                                start=(fi == 0),
                                stop=(not dense and fi == KH - 1))
                        if dense:
                            for fi in range(KH):
                                nc.tensor.matmul(
                                    out=iou_ps[:], lhsT=uiou_t[fi][fo][:],
                                    rhs=htg[:, fi, hf * 512:(hf + 1) * 512],
                                    start=False, stop=(fi == KH - 1))
                        nc.scalar.activation(
                            out=sb_act[fo][:, hf * 512:(hf + 1) * 512],
                            in_=iou_ps[:],
                            func=(SIG if fo < 2 * KH else TANH),
                            bias=bcol_t[:, fo:fo + 1])

                cn3 = gp.tile([128, KH, GN], bf16, tag="cn3")
                hn3 = gp.tile([128, KH, GN], bf16, tag="hn3")
                for fi in range(KH):
                    i_sb, o_sb, u_sb = (sb_act[fi], sb_act[KH + fi],
                                        sb_act[2 * KH + fi])
                    if dense:
                        iu = wp.tile([128, GN], bf16, tag="iu")
                        nc.gpsimd.tensor_tensor(out=iu[:], in0=i_sb[:],
                                                in1=u_sb[:], op=MUL)
                        nc.vector.tensor_tensor(out=cn3[:, fi, :], in0=iu[:],
                                                in1=cag[:, fi, :], op=ADD)
                    else:
                        nc.gpsimd.tensor_tensor(out=cn3[:, fi, :], in0=i_sb[:],
                                                in1=u_sb[:], op=MUL)
                    th = wp.tile([128, GN], bf16, tag="th")
                    nc.scalar.activation(out=th[:], in_=cn3[:, fi, :], func=TANH)
                    nc.gpsimd.tensor_tensor(out=hn3[:, fi, :], in0=o_sb[:],
                                            in1=th[:], op=MUL)
                nc.gpsimd.dma_start(
                    out=coutT_d[:, g * GN:(g + 1) * GN].rearrange(
                        "(f p) c -> p f c", p=128), in_=cn3[:])
                nc.gpsimd.dma_start(
                    out=houtT_d[:, g * GN:(g + 1) * GN].rearrange(
                        "(f p) c -> p f c", p=128), in_=hn3[:])

    nc.compile()
    return nc


def _pack_blocks(deg, nblk, caps_e):
    """Balanced bin packing: place nodes (desc degree) into blocks of BLK
    node slots, respecting per-block edge capacities where possible."""
    npc = deg.shape[0]
    order = np.argsort(-deg, kind="stable")
    rem_e = caps_e.astype(np.int64).copy()
    rem_n = np.full(nblk, BLK, np.int64)
    heap = [(-rem_e[b], b) for b in range(nblk)]
    heapq.heapify(heap)
    assign = np.empty(npc, np.int64)
    for nd in order:
        d = int(deg[nd])
        tmp = []
        placed = False
        while heap:
            negre, b = heapq.heappop(heap)
            if rem_n[b] == 0:
                continue
            if d <= rem_e[b] or d == 0:
                rem_e[b] -= d
                rem_n[b] -= 1
                assign[nd] = b
                placed = True
                if rem_n[b] > 0:
                    heapq.heappush(heap, (-rem_e[b], b))
                break
            tmp.append((negre, b))
        for t in tmp:
            heapq.heappush(heap, t)
        if not placed:
            b = int(np.argmax(np.where(rem_n > 0, rem_e, -(1 << 60))))
            rem_e[b] -= d
            rem_n[b] -= 1
            assign[nd] = b
    return assign


def _prep_inputs(x, h, c, W_iou, U_iou, b_iou, U_f_w, U_f_b,
                 edge_src, edge_dst):
    n = x.shape[0]
    assert n % N_CORES == 0
    npc = n // N_CORES
    nblk = -(-npc // GN) * G  # groups of G blocks covering npc nodes
    loc = nblk * BLK

    x = np.asarray(x, np.float32)
    h = np.asarray(h, np.float32)
    c = np.asarray(c, np.float32)
    edge_src = np.asarray(edge_src, np.int64)
    edge_dst = np.asarray(edge_dst, np.int64)

    owner = edge_dst // npc
    per_core = []
    for k in range(N_CORES):
        m = owner == k
        ldst = edge_dst[m] - k * npc
        src = edge_src[m]
        deg = np.bincount(ldst, minlength=npc)
        per_core.append((src, ldst, deg))

    E_max = max(src.shape[0] for src, _, _ in per_core)
    nd_max = max(int((deg > 0).sum()) for _, _, deg in per_core)
    D8 = min(G * (-(-(-(-nd_max // BLK)) // G)), nblk)
    # edge capacity: n2 blocks with 256-edge budget, rest 128
    n2 = int(np.clip(-(-(E_max + 192 - 128 * D8) // 128), 0, D8))
    caps_e = np.array([256] * n2 + [128] * (D8 - n2), np.int64)

    # per-core packing of nonzero-degree nodes into dense blocks
    cores = []
    ec_arr = np.zeros((N_CORES, D8), np.int64)
    for k in range(N_CORES):
        src, ldst, deg = per_core[k]
        nz = np.flatnonzero(deg > 0)
        assign_nz = _pack_blocks(deg[nz], D8, caps_e)
        blk_of = np.full(npc, -1, np.int64)
        blk_of[nz] = assign_nz
        # fill remaining slots (dense leftovers first, then zero blocks)
        used = np.bincount(assign_nz, minlength=nblk)
        free_slots = BLK - used
        zeros_ = np.flatnonzero(deg == 0)
        fill_blocks = np.repeat(np.arange(nblk), free_slots)
        blk_of[zeros_] = fill_blocks[:zeros_.size]
        # position within block
        order = np.argsort(blk_of, kind="stable")
        cnt_all = np.bincount(blk_of, minlength=nblk)
        starts = np.concatenate([[0], np.cumsum(cnt_all)[:-1]])
        pos = np.empty(npc, np.int64)
        pos[order] = np.arange(npc) - np.repeat(starts, cnt_all)
        col_of = blk_of * BLK + pos
        # edge counts per dense block
        ecnt = np.bincount(blk_of[ldst], minlength=D8)[:D8]
        ec_arr[k] = -(-np.maximum(ecnt, 1) // 128)
        cores.append((src, ldst, col_of, blk_of))
    ec_list = tuple(int(v) for v in ec_arr.max(axis=0))
    ecs = np.asarray(ec_list)
    ecb = np.concatenate([[0], np.cumsum(ecs)])
    tot_ch = int(ecb[-1])
    tot_e = tot_ch * 128

    groups, htot, ctot = _group_meta(ec_list, D8)

    ufwT = np.ascontiguousarray(np.asarray(U_f_w, np.float32).T).astype(BF)
    wiouT = np.ascontiguousarray(np.asarray(W_iou, np.float32).T).astype(BF)
    uiouT = np.ascontiguousarray(np.asarray(U_iou, np.float32).T).astype(BF)
    b_iou_f = np.asarray(b_iou, np.float32).reshape(-1)
    bcol = np.ascontiguousarray(b_iou_f.reshape(FO, 128).T)
    iota = np.broadcast_to(np.arange(BLK, dtype=np.float32),
                           (128, BLK)).astype(BF)
    U_f_b_f = np.asarray(U_f_b, np.float32).reshape(-1)
    fbias_zero = not U_f_b_f.any()

    in_maps = []
    col_maps = []
    for k in range(N_CORES):
        src, ldst, col_of, blk_of = cores[k]
        eblk = blk_of[ldst]
        eorder = np.argsort(eblk, kind="stable")
        cnt = np.bincount(eblk, minlength=D8)[:D8]
        start = np.zeros(D8, np.int64)
        np.cumsum(cnt[:-1], out=start[1:])
        slot_in_blk = np.arange(eblk.size) - start[eblk[eorder]]
        flat_pos = ecb[eblk[eorder]] * 128 + slot_in_blk
        hch = np.zeros((tot_e, H), np.float32)
        cch = np.zeros((tot_e, H), np.float32)
        hch[flat_pos] = h[src[eorder]]
        cch[flat_pos] = c[src[eorder]]
        flat_dst = np.full(tot_e, -1.0, np.float32)
        flat_dst[flat_pos] = (col_of[ldst[eorder]] % BLK).astype(np.float32)

        hslab = np.empty((128, htot), BF)
        cslab = np.empty((128, ctot), BF)
        drel = np.ascontiguousarray(
            flat_dst.reshape(tot_ch, 128).T)  # [128(e), chunk]
        for gm, g in zip(groups, range(len(groups))):
            nch = gm["nch"]
            e0 = int(ecb[g * G]) * 128
            hb = hch[e0:e0 + nch * 128]          # [nch*128, H]
            # edge-major: [p(edge), chunk, feat]
            p1 = hb.reshape(nch, 128, H).transpose(1, 0, 2).reshape(128, nch * H)
            # feature-major per chunk: [p(feat), chunk, fi, e]
            p2 = (hb.reshape(nch, 128, KH, 128)   # [ch, e, fi, fp]
                  .transpose(3, 0, 2, 1)          # [fp, ch, fi, e]
                  .reshape(128, nch * 256))
            hslab[:, gm["hbase"]:gm["hbase"] + nch * 256] = p1.astype(BF)
            hslab[:, gm["hbase"] + nch * 256:
                  gm["hbase"] + nch * 512] = p2.astype(BF)
            cb = cch[e0:e0 + nch * 128]
            q1 = cb.reshape(nch, 128, H).transpose(1, 0, 2).reshape(128, nch * H)
            cslab[:, gm["cbase"]:gm["cbase"] + nch * 256] = q1.astype(BF)

        xT = np.zeros((X, loc), np.float32)
        xT[:, col_of] = x[k * npc:(k + 1) * npc].T
        im = {
            "hslab": hslab, "cslab": cslab, "drel": drel, "xT": xT.astype(BF),
            "ufwT": ufwT, "wiouT": wiouT, "uiouT": uiouT,
            "bcol": bcol, "iota": iota,
        }
        if not fbias_zero:
            im["ufb"] = U_f_b_f.reshape(1, H).astype(BF)
            im["ones"] = np.ones((1, 128), BF)
        in_maps.append(im)
        col_maps.append(col_of)

    meta = dict(n=n, npc=npc, nblk=nblk, D8=D8, loc=loc, ec_list=ec_list,
                fbias_zero=fbias_zero, col_maps=col_maps)
    return in_maps, meta


def kernel(x, h, c, W_iou, U_iou, b_iou, U_f_w, U_f_b, edge_src, edge_dst,
           _trace=False):
    global LAST_EXEC_TIME_NS
    in_maps, meta = _prep_inputs(x, h, c, W_iou, U_iou, b_iou, U_f_w, U_f_b,
                                 edge_src, edge_dst)
    key = (meta["nblk"], meta["D8"], meta["loc"], meta["ec_list"],
           meta["fbias_zero"])
    nc = _PROGRAM_CACHE.get(key)
    if nc is None:
        nc = _build_program(meta["nblk"], meta["D8"], meta["loc"],
                            meta["ec_list"], meta["fbias_zero"])
        _PROGRAM_CACHE[key] = nc
    if not _trace:
        os.environ.setdefault("BASS_NEVER_TRACE", "1")
    res = run_bass_kernel_spmd(nc, in_maps, list(range(N_CORES)),
                               trace=_trace, trace_cores=[0] if _trace else None)
    if _trace:
        LAST_EXEC_TIME_NS = res.exec_time_ns

    n = meta["n"]
    npc = meta["npc"]
    h_new = np.empty((n, H), np.float32)
    c_new = np.empty((n, H), np.float32)
    for k in range(N_CORES):
        cols = meta["col_maps"][k]
        h_new[k * npc:(k + 1) * npc] = \
            np.asarray(res.results[k]["houtT"], BF)[:, cols].T.astype(np.float32)
        c_new[k * npc:(k + 1) * npc] = \
            np.asarray(res.results[k]["coutT"], BF)[:, cols].T.astype(np.float32)
    return h_new, c_new


# revision 13
# speedup vs baseline: 1.2256x; 1.0714x over previous
"""ChildSum TreeLSTM cell for 8 Trainium2 NeuronCores — self-contained kernel.

Sharding: nodes and edges partitioned by edge_dst owner across 8 cores
(25000 nodes each). Within a core, nodes are permuted into 200 blocks of
128 destination nodes. Nodes with zero in-degree (~37%, Poisson degree)
are segregated into trailing "zero" blocks whose groups skip the U-matmul
and aggregation entirely (iou = W x only, c_new = i*u). Dense blocks are
bin-packed so block edge counts land near 128/256 (1-2 chunks of 128
edges). The host stages the halo — h[src]/c[src] rows per edge in block
order, h[src] both edge-major and feature-major — in bf16, so the device
kernel is streaming DMA + bf16 matmuls (PSUM accumulates in f32).

Device pipeline per group of 8 blocks (1024 destination nodes):
  per chunk pair: f = sigmoid(h_childT.T @ U_f)        (PE + ACT)
                  fc = f * c_child                      (DVE)
  per chunk:      S = one-hot(dst) via iota/is_equal    (DVE)
                  h_tildT += h_child.T @ S              (PE, PSUM)
                  c_aggT  += fc.T @ S                   (PE, PSUM)
  per subround (4 blocks): copy PSUM -> bf16 SBUF       (DVE)
  iou halves:     iouT = W.T@xT [+ U.T@h_tildT]         (PE)
                  i,o = sigmoid, u = tanh               (ACT)
  apply:          iu = i*u (GpSimd); cn = iu + c_agg (DVE)
                  th = tanh(cn) (ACT); hn = o*th (GpSimd)
Outputs return feature-major and permuted; the host inverts both.
"""
import os
import sys

for _p in ("/opt/trn_rl_repo",):
    if _p not in sys.path:
        sys.path.insert(0, _p)

import heapq

import numpy as np
import ml_dtypes

import concourse.bass as bass
import concourse.bacc as bacc
import concourse.mybir as mybir
import concourse.tile as tile
from concourse.bass_utils import run_bass_kernel_spmd

f32 = mybir.dt.float32
bf16 = mybir.dt.bfloat16
BF = ml_dtypes.bfloat16

N_CORES = 8
BLK = 128    # destination nodes per block
G = 8        # blocks per group
GN = G * BLK # nodes per group
SUB = 4      # blocks per scatter subround
H = 256
X = 256
KH = H // 128   # 2
FO = 3 * H // 128  # 6

LAST_EXEC_TIME_NS = None
_PROGRAM_CACHE = {}


def _group_meta(ec_list, D8):
    """Per dense group: chunk table + slab offsets (shared across cores)."""
    ecs = list(ec_list)
    groups = []
    hbase = 0
    cbase = 0
    for g in range(D8 // G):
        blocks = ecs[g * G:(g + 1) * G]
        nch = sum(blocks)
        chunks = []  # (gchunk, block_local, cib, ec_of_block)
        gc = 0
        for bl, ec in enumerate(blocks):
            for c in range(ec):
                chunks.append((gc, bl, c, ec))
                gc += 1
        groups.append(dict(nch=nch, chunks=chunks, hbase=hbase, cbase=cbase,
                           dbase=sum(ecs[:g * G])))
        hbase += nch * 512
        cbase += nch * 256
    return groups, hbase, cbase


def _build_program(nblk, D8, loc, ec_list, fbias_zero):
    groups, htot, ctot = _group_meta(ec_list, D8)
    ngroups = nblk // G
    dense_groups = D8 // G
    max_h = max(g["nch"] for g in groups) * 512
    max_c = max(g["nch"] for g in groups) * 256
    max_n = max(g["nch"] for g in groups)
    tot_ch = sum(ec_list)

    nc = bacc.Bacc(None, target_bir_lowering=False, debug=False)

    hsl_d = nc.declare_dram_parameter("hslab", [128, htot], bf16, isOutput=False)
    csl_d = nc.declare_dram_parameter("cslab", [128, ctot], bf16, isOutput=False)
    drel_d = nc.declare_dram_parameter("drel", [128, tot_ch], f32, isOutput=False)
    xT_d = nc.declare_dram_parameter("xT", [X, loc], bf16, isOutput=False)
    ufwT_d = nc.declare_dram_parameter("ufwT", [X, H], bf16, isOutput=False)
    wiouT_d = nc.declare_dram_parameter("wiouT", [X, 3 * H], bf16, isOutput=False)
    uiouT_d = nc.declare_dram_parameter("uiouT", [H, 3 * H], bf16, isOutput=False)
    bcol_d = nc.declare_dram_parameter("bcol", [128, FO], f32, isOutput=False)
    iota_d = nc.declare_dram_parameter("iota", [128, BLK], bf16, isOutput=False)
    if not fbias_zero:
        ufb_d = nc.declare_dram_parameter("ufb", [1, H], bf16, isOutput=False)
        ones_d = nc.declare_dram_parameter("ones", [1, 128], bf16, isOutput=False)

    houtT_d = nc.declare_dram_parameter("houtT", [H, loc], bf16, isOutput=True)
    coutT_d = nc.declare_dram_parameter("coutT", [H, loc], bf16, isOutput=True)

    SIG = mybir.ActivationFunctionType.Sigmoid
    TANH = mybir.ActivationFunctionType.Tanh
    MUL = mybir.AluOpType.mult
    ADD = mybir.AluOpType.add

    with tile.TileContext(nc) as tc:
        with (
            tc.tile_pool(name="const", bufs=1) as cpool,
            tc.tile_pool(name="io", bufs=2) as iop,
            tc.tile_pool(name="work", bufs=3) as wp,
            tc.tile_pool(name="grp", bufs=2) as gp,
            tc.tile_pool(name="ps_f", bufs=2, space="PSUM") as psp,
            tc.tile_pool(name="ps_iou", bufs=2, space="PSUM") as psi,
            tc.tile_pool(name="ps_acc", bufs=1, space="PSUM") as pacc,
        ):
            iota_t = cpool.tile([128, BLK], bf16)
            nc.sync.dma_start(out=iota_t[:], in_=iota_d[:])
            bcol_t = cpool.tile([128, FO], f32)
            nc.sync.dma_start(out=bcol_t[:], in_=bcol_d[:])
            ufw_t = []
            for fi in range(KH):
                t = cpool.tile([128, H], bf16, tag=f"ufw{fi}", name=f"ufw{fi}")
                nc.sync.dma_start(out=t[:], in_=ufwT_d[fi * 128:(fi + 1) * 128, :])
                ufw_t.append(t)
            wiou_t = [[None] * FO for _ in range(KH)]
            uiou_t = [[None] * FO for _ in range(KH)]
            for fi in range(KH):
                for fo in range(FO):
                    t = cpool.tile([128, 128], bf16, tag=f"wiou{fi}_{fo}",
                                   name=f"wiou{fi}_{fo}")
                    nc.sync.dma_start(
                        out=t[:], in_=wiouT_d[fi * 128:(fi + 1) * 128,
                                              fo * 128:(fo + 1) * 128])
                    wiou_t[fi][fo] = t
                    t = cpool.tile([128, 128], bf16, tag=f"uiou{fi}_{fo}",
                                   name=f"uiou{fi}_{fo}")
                    nc.sync.dma_start(
                        out=t[:], in_=uiouT_d[fi * 128:(fi + 1) * 128,
                                              fo * 128:(fo + 1) * 128])
                    uiou_t[fi][fo] = t
            if not fbias_zero:
                ones_t = cpool.tile([1, 128], bf16)
                nc.sync.dma_start(out=ones_t[:], in_=ones_d[:])
                ufb_t = cpool.tile([1, H], bf16)
                nc.sync.dma_start(out=ufb_t[:], in_=ufb_d[:])

            # interleave zero groups among dense groups so the PE never
            # sits idle through an activation-only stretch
            nz_g = ngroups - dense_groups
            keys = [((i + 0.5) / max(dense_groups, 1), i)
                    for i in range(dense_groups)]
            keys += [((j + 0.5) / max(nz_g, 1), dense_groups + j)
                     for j in range(nz_g)]
            for _, g in sorted(keys):
                dense = g < dense_groups
                xtg = gp.tile([128, KH, GN], bf16, tag="xtg")
                nc.sync.dma_start(
                    out=xtg[:],
                    in_=xT_d[:, g * GN:(g + 1) * GN].rearrange(
                        "(f p) c -> p f c", p=128))

                if dense:
                    gm = groups[g]
                    nch = gm["nch"]
                    hsl_t = iop.tile([128, max_h], bf16, tag="hsl")
                    nc.sync.dma_start(
                        out=hsl_t[:, :nch * 512],
                        in_=hsl_d[:, gm["hbase"]:gm["hbase"] + nch * 512])
                    csl_t = iop.tile([128, max_c], bf16, tag="csl")
                    nc.sync.dma_start(
                        out=csl_t[:, :nch * 256],
                        in_=csl_d[:, gm["cbase"]:gm["cbase"] + nch * 256])
                    drel_t = iop.tile([128, max_n], f32, tag="drel")
                    nc.sync.dma_start(
                        out=drel_t[:, :nch],
                        in_=drel_d[:, gm["dbase"]:gm["dbase"] + nch])
                    htg = gp.tile([128, KH, GN], bf16, tag="htg")
                    cag = gp.tile([128, KH, GN], bf16, tag="cag")
                    HT0 = nch * 256  # h_childT section offset

                    for sub in range(2):
                        L = [ch for ch in gm["chunks"] if ch[1] // SUB == sub]
                        ht_ps = [pacc.tile([128, SUB * BLK], f32, tag=f"ht{fi}",
                                           name=f"ht_ps{fi}")
                                 for fi in range(KH)]
                        ca_ps = [pacc.tile([128, SUB * BLK], f32, tag=f"ca{fi}",
                                           name=f"ca_ps{fi}")
                                 for fi in range(KH)]
                        for p0 in range(0, len(L), 2):
                            pair = L[p0:p0 + 2]
                            w = 256 * len(pair)
                            gc0 = pair[0][0]
                            f_ps = psp.tile([128, 512], f32, tag="f")
                            for q, (gc, bl, cib, ec) in enumerate(pair):
                                for fi in range(KH):
                                    nc.tensor.matmul(
                                        out=f_ps[:, q * 256:(q + 1) * 256],
                                        lhsT=hsl_t[:, HT0 + gc * 256 + fi * 128:
                                                   HT0 + gc * 256 + fi * 128 + 128],
                                        rhs=ufw_t[fi][:],
                                        start=(fi == 0),
                                        stop=(fi == KH - 1 and fbias_zero))
                                if not fbias_zero:
                                    nc.tensor.matmul(
                                        out=f_ps[:, q * 256:(q + 1) * 256],
                                        lhsT=ones_t[:], rhs=ufb_t[:],
                                        start=False, stop=True)
                            f_sb = wp.tile([128, 512], bf16, tag="fsb")
                            nc.scalar.activation(out=f_sb[:, :w],
                                                 in_=f_ps[:, :w], func=SIG)
                            fc = wp.tile([128, 512], bf16, tag="fc")
                            nc.vector.tensor_tensor(
                                out=fc[:, :w], in0=f_sb[:, :w],
                                in1=csl_t[:, gc0 * 256:gc0 * 256 + w], op=MUL)
                            for q, (gc, bl, cib, ec) in enumerate(pair):
                                s_t = wp.tile([128, BLK], bf16, tag="S")
                                nc.vector.tensor_scalar(
                                    out=s_t[:], in0=iota_t[:],
                                    scalar1=drel_t[:, gc:gc + 1],
                                    scalar2=None,
                                    op0=mybir.AluOpType.is_equal)
                                bl4 = bl % SUB
                                first = (cib == 0)
                                last = (cib == ec - 1)
                                for fi in range(KH):
                                    nc.tensor.matmul(
                                        out=ht_ps[fi][:, bl4 * BLK:(bl4 + 1) * BLK],
                                        lhsT=hsl_t[:, gc * 256 + fi * 128:
                                                   gc * 256 + fi * 128 + 128],
                                        rhs=s_t[:], start=first, stop=last)
                                    nc.tensor.matmul(
                                        out=ca_ps[fi][:, bl4 * BLK:(bl4 + 1) * BLK],
                                        lhsT=fc[:, q * 256 + fi * 128:
                                                q * 256 + fi * 128 + 128],
                                        rhs=s_t[:], start=first, stop=last)
                        SW = SUB * BLK
                        for fi in range(KH):
                            nc.vector.tensor_copy(
                                out=htg[:, fi, sub * SW:(sub + 1) * SW],
                                in_=ht_ps[fi][:])
                            nc.vector.tensor_copy(
                                out=cag[:, fi, sub * SW:(sub + 1) * SW],
                                in_=ca_ps[fi][:])

                # ---- iou for the whole group, in 512-node halves ----
                sb_act = [wp.tile([128, GN], bf16, tag=f"act{fo}",
                                  name=f"act{fo}") for fo in range(FO)]
                for fo in range(FO):
                    for hf in range(2):
                        iou_ps = psi.tile([128, 512], f32, tag="iou")
                        for fi in range(KH):
                            nc.tensor.matmul(
                                out=iou_ps[:], lhsT=wiou_t[fi][fo][:],
                                rhs=xtg[:, fi, hf * 512:(hf + 1) * 512],
                                start=(fi == 0),
                                stop=(not dense and fi == KH - 1))
                        if dense:
                            for fi in range(KH):
                                nc.tensor.matmul(
                                    out=iou_ps[:], lhsT=uiou_t[fi][fo][:],
                                    rhs=htg[:, fi, hf * 512:(hf + 1) * 512],
                                    start=False, stop=(fi == KH - 1))
                        nc.scalar.activation(
                            out=sb_act[fo][:, hf * 512:(hf + 1) * 512],
                            in_=iou_ps[:],
                            func=(SIG if fo < 2 * KH else TANH),
                            bias=bcol_t[:, fo:fo + 1])

                cn3 = gp.tile([128, KH, GN], bf16, tag="cn3")
                hn3 = gp.tile([128, KH, GN], bf16, tag="hn3")
                for fi in range(KH):
                    i_sb, o_sb, u_sb = (sb_act[fi], sb_act[KH + fi],
                                        sb_act[2 * KH + fi])
                    if dense:
                        iu = wp.tile([128, GN], bf16, tag="iu")
                        nc.gpsimd.tensor_tensor(out=iu[:], in0=i_sb[:],
                                                in1=u_sb[:], op=MUL)
                        nc.vector.tensor_tensor(out=cn3[:, fi, :], in0=iu[:],
                                                in1=cag[:, fi, :], op=ADD)
                    else:
                        nc.gpsimd.tensor_tensor(out=cn3[:, fi, :], in0=i_sb[:],
                                                in1=u_sb[:], op=MUL)
                    th = wp.tile([128, GN], bf16, tag="th")
                    nc.scalar.activation(out=th[:], in_=cn3[:, fi, :], func=TANH)
                    nc.gpsimd.tensor_tensor(out=hn3[:, fi, :], in0=o_sb[:],
                                            in1=th[:], op=MUL)
                nc.gpsimd.dma_start(
                    out=coutT_d[:, g * GN:(g + 1) * GN].rearrange(
                        "(f p) c -> p f c", p=128), in_=cn3[:])
                nc.gpsimd.dma_start(
                    out=houtT_d[:, g * GN:(g + 1) * GN].rearrange(
                        "(f p) c -> p f c", p=128), in_=hn3[:])

    nc.compile()
    return nc


def _pack_blocks(deg_nz, D8, n2):
    """Pack nonzero-degree nodes into D8 blocks of BLK node slots.
    Blocks [0, n2) target 256 edges (2 chunks), [n2, D8) target 128
    (1 chunk). Heavy nodes (deg>=2) are snake-dealt across the first n2
    blocks, topped off with degree-1 nodes; remaining degree-1 nodes are
    dealt evenly over the 1-chunk blocks. Overflow (rare) lands in the
    emptiest block — ec is recomputed from actual counts so it stays
    correct, just costs an extra chunk."""
    nnz = deg_nz.shape[0]
    assign = np.full(nnz, -1, np.int64)
    heavy = np.flatnonzero(deg_nz >= 2)
    ones = np.flatnonzero(deg_nz == 1)
    o = np.argsort(-deg_nz[heavy], kind="stable")
    heavy = heavy[o]
    caps = np.concatenate([np.full(n2, 2 * BLK), np.full(D8 - n2, BLK)])
    rem_e = caps.astype(np.int64).copy()
    rem_n = np.full(D8, BLK, np.int64)
    nb = max(n2, 1)
    for i in range(heavy.shape[0]):
        r, j = divmod(i, nb)
        if r % 2:
            j = nb - 1 - j
        j = min(j, D8 - 1)
        assign[heavy[i]] = j
        rem_e[j] -= deg_nz[heavy[i]]
        rem_n[j] -= 1
    # top off 2-chunk blocks with degree-1 nodes
    p = 0
    for j in range(n2):
        t = int(max(0, min(rem_e[j], rem_n[j])))
        t = min(t, ones.shape[0] - p)
        if t > 0:
            assign[ones[p:p + t]] = j
            rem_e[j] -= t
            rem_n[j] -= t
            p += t
    # deal remaining degree-1 nodes evenly over 1-chunk blocks
    rest = ones[p:]
    nb1 = D8 - n2
    if rest.size and nb1 > 0:
        take = np.minimum(
            np.full(nb1, -(-rest.size // nb1), np.int64), BLK)
        take = np.minimum(take, rem_n[n2:])
        csum = np.cumsum(take)
        total = min(int(csum[-1]), rest.size)
        blocks = np.repeat(np.arange(n2, D8), take)[:total]
        assign[rest[:total]] = blocks
        np.subtract.at(rem_e, blocks, 1)
        np.subtract.at(rem_n, blocks, 1)
        rest = rest[total:]
    # overflow: emptiest-by-edges block with free slots
    for nd in rest:
        j = int(np.argmax(np.where(rem_n > 0, rem_e, -(1 << 60))))
        assign[nd] = j
        rem_e[j] -= 1
        rem_n[j] -= 1
    return assign


def _prep_inputs(x, h, c, W_iou, U_iou, b_iou, U_f_w, U_f_b,
                 edge_src, edge_dst):
    n = x.shape[0]
    assert n % N_CORES == 0
    npc = n // N_CORES
    nblk = -(-npc // GN) * G  # groups of G blocks covering npc nodes
    loc = nblk * BLK

    x = np.asarray(x, np.float32)
    h = np.asarray(h, np.float32)
    c = np.asarray(c, np.float32)
    edge_src = np.asarray(edge_src, np.int64)
    edge_dst = np.asarray(edge_dst, np.int64)

    owner = edge_dst // npc
    per_core = []
    for k in range(N_CORES):
        m = owner == k
        ldst = edge_dst[m] - k * npc
        src = edge_src[m]
        deg = np.bincount(ldst, minlength=npc)
        per_core.append((src, ldst, deg))

    nd_max = max(int((deg > 0).sum()) for _, _, deg in per_core)
    D8 = min(G * (-(-(-(-nd_max // BLK)) // G)), nblk)
    # minimal feasible count of 2-chunk blocks (see _pack_blocks)
    n2 = 1
    for _, _, deg in per_core:
        dn = deg[deg > 0]
        nH = int((dn >= 2).sum())
        EH = int(dn[dn >= 2].sum())
        nd1 = int((dn == 1).sum())
        b = max(1, -(-EH // 250))
        while b < D8:
            hj = -(-EH // b)
            nhj = -(-nH // b)
            tj = max(0, min(2 * BLK - hj, BLK - nhj))
            if nd1 - b * tj <= (D8 - b) * BLK and hj <= 2 * BLK - 6:
                break
            b += 1
        n2 = max(n2, b)

    # per-core packing of nonzero-degree nodes into dense blocks
    cores = []
    ec_arr = np.zeros((N_CORES, D8), np.int64)
    for k in range(N_CORES):
        src, ldst, deg = per_core[k]
        nz = np.flatnonzero(deg > 0)
        assign_nz = _pack_blocks(deg[nz], D8, n2)
        blk_of = np.full(npc, -1, np.int64)
        blk_of[nz] = assign_nz
        # fill remaining slots (dense leftovers first, then zero blocks)
        used = np.bincount(assign_nz, minlength=nblk)
        free_slots = BLK - used
        zeros_ = np.flatnonzero(deg == 0)
        fill_blocks = np.repeat(np.arange(nblk), free_slots)
        blk_of[zeros_] = fill_blocks[:zeros_.size]
        # position within block
        order = np.argsort(blk_of, kind="stable")
        cnt_all = np.bincount(blk_of, minlength=nblk)
        starts = np.concatenate([[0], np.cumsum(cnt_all)[:-1]])
        pos = np.empty(npc, np.int64)
        pos[order] = np.arange(npc) - np.repeat(starts, cnt_all)
        col_of = blk_of * BLK + pos
        # edge counts per dense block
        ecnt = np.bincount(blk_of[ldst], minlength=D8)[:D8]
        ec_arr[k] = -(-np.maximum(ecnt, 1) // 128)
        cores.append((src, ldst, col_of, blk_of))
    ec_list = tuple(int(v) for v in ec_arr.max(axis=0))
    ecs = np.asarray(ec_list)
    ecb = np.concatenate([[0], np.cumsum(ecs)])
    tot_ch = int(ecb[-1])
    tot_e = tot_ch * 128

    groups, htot, ctot = _group_meta(ec_list, D8)

    ufwT = np.ascontiguousarray(np.asarray(U_f_w, np.float32).T).astype(BF)
    wiouT = np.ascontiguousarray(np.asarray(W_iou, np.float32).T).astype(BF)
    uiouT = np.ascontiguousarray(np.asarray(U_iou, np.float32).T).astype(BF)
    b_iou_f = np.asarray(b_iou, np.float32).reshape(-1)
    bcol = np.ascontiguousarray(b_iou_f.reshape(FO, 128).T)
    iota = np.broadcast_to(np.arange(BLK, dtype=np.float32),
                           (128, BLK)).astype(BF)
    U_f_b_f = np.asarray(U_f_b, np.float32).reshape(-1)
    fbias_zero = not U_f_b_f.any()

    in_maps = []
    col_maps = []
    for k in range(N_CORES):
        src, ldst, col_of, blk_of = cores[k]
        eblk = blk_of[ldst]
        eorder = np.argsort(eblk, kind="stable")
        cnt = np.bincount(eblk, minlength=D8)[:D8]
        start = np.zeros(D8, np.int64)
        np.cumsum(cnt[:-1], out=start[1:])
        slot_in_blk = np.arange(eblk.size) - start[eblk[eorder]]
        flat_pos = ecb[eblk[eorder]] * 128 + slot_in_blk
        hch = np.zeros((tot_e, H), np.float32)
        cch = np.zeros((tot_e, H), np.float32)
        hch[flat_pos] = h[src[eorder]]
        cch[flat_pos] = c[src[eorder]]
        flat_dst = np.full(tot_e, -1.0, np.float32)
        flat_dst[flat_pos] = (col_of[ldst[eorder]] % BLK).astype(np.float32)

        hslab = np.empty((128, htot), BF)
        cslab = np.empty((128, ctot), BF)
        drel = np.ascontiguousarray(
            flat_dst.reshape(tot_ch, 128).T)  # [128(e), chunk]
        for gm, g in zip(groups, range(len(groups))):
            nch = gm["nch"]
            e0 = int(ecb[g * G]) * 128
            hb = hch[e0:e0 + nch * 128]          # [nch*128, H]
            # edge-major: [p(edge), chunk, feat]
            p1 = hb.reshape(nch, 128, H).transpose(1, 0, 2).reshape(128, nch * H)
            # feature-major per chunk: [p(feat), chunk, fi, e]
            p2 = (hb.reshape(nch, 128, KH, 128)   # [ch, e, fi, fp]
                  .transpose(3, 0, 2, 1)          # [fp, ch, fi, e]
                  .reshape(128, nch * 256))
            hslab[:, gm["hbase"]:gm["hbase"] + nch * 256] = p1.astype(BF)
            hslab[:, gm["hbase"] + nch * 256:
                  gm["hbase"] + nch * 512] = p2.astype(BF)
            cb = cch[e0:e0 + nch * 128]
            q1 = cb.reshape(nch, 128, H).transpose(1, 0, 2).reshape(128, nch * H)
            cslab[:, gm["cbase"]:gm["cbase"] + nch * 256] = q1.astype(BF)

        xT = np.zeros((X, loc), np.float32)
        xT[:, col_of] = x[k * npc:(k + 1) * npc].T
        im = {
            "hslab": hslab, "cslab": cslab, "drel": drel, "xT": xT.astype(BF),
            "ufwT": ufwT, "wiouT": wiouT, "uiouT": uiouT,
            "bcol": bcol, "iota": iota,
        }
        if not fbias_zero:
            im["ufb"] = U_f_b_f.reshape(1, H).astype(BF)
            im["ones"] = np.ones((1, 128), BF)
        in_maps.append(im)
        col_maps.append(col_of)

    meta = dict(n=n, npc=npc, nblk=nblk, D8=D8, loc=loc, ec_list=ec_list,
                fbias_zero=fbias_zero, col_maps=col_maps)
    return in_maps, meta


def kernel(x, h, c, W_iou, U_iou, b_iou, U_f_w, U_f_b, edge_src, edge_dst,
           _trace=False):
    global LAST_EXEC_TIME_NS
    in_maps, meta = _prep_inputs(x, h, c, W_iou, U_iou, b_iou, U_f_w, U_f_b,
                                 edge_src, edge_dst)
    key = (meta["nblk"], meta["D8"], meta["loc"], meta["ec_list"],
           meta["fbias_zero"])
    nc = _PROGRAM_CACHE.get(key)
    if nc is None:
        nc = _build_program(meta["nblk"], meta["D8"], meta["loc"],
                            meta["ec_list"], meta["fbias_zero"])
        _PROGRAM_CACHE[key] = nc
    if not _trace:
        os.environ.setdefault("BASS_NEVER_TRACE", "1")
    res = run_bass_kernel_spmd(nc, in_maps, list(range(N_CORES)),
                               trace=_trace, trace_cores=[0] if _trace else None)
    if _trace:
        LAST_EXEC_TIME_NS = res.exec_time_ns

    n = meta["n"]
    npc = meta["npc"]
    h_new = np.empty((n, H), np.float32)
    c_new = np.empty((n, H), np.float32)
    for k in range(N_CORES):
        cols = meta["col_maps"][k]
        h_new[k * npc:(k + 1) * npc] = \
            np.asarray(res.results[k]["houtT"], BF)[:, cols].T.astype(np.float32)
        c_new[k * npc:(k + 1) * npc] = \
            np.asarray(res.results[k]["coutT"], BF)[:, cols].T.astype(np.float32)
    return h_new, c_new


# revision 28
# speedup vs baseline: 1.2760x; 1.0411x over previous
"""ChildSum TreeLSTM cell for 8 Trainium2 NeuronCores — self-contained kernel.

Sharding: nodes and edges partitioned by edge_dst owner across 8 cores
(25000 nodes each). Within a core, nodes are permuted into 200 blocks of
128 destination nodes. Nodes with zero in-degree (~37%, Poisson degree)
are segregated into trailing "zero" blocks whose groups skip the U-matmul
and aggregation entirely (iou = W x only, c_new = i*u). Dense blocks are
bin-packed so block edge counts land near 128/256 (1-2 chunks of 128
edges). The host stages the halo — h[src]/c[src] rows per edge in block
order, h[src] both edge-major and feature-major — in bf16, so the device
kernel is streaming DMA + bf16 matmuls (PSUM accumulates in f32).

Device pipeline per group of 8 blocks (1024 destination nodes):
  per chunk pair: f = sigmoid(h_childT.T @ U_f)        (PE + ACT)
                  fc = f * c_child                      (DVE)
                  h_tildT += h_child.T @ S              (PE, PSUM)
                  c_aggT  += fc.T @ S                   (PE, PSUM)
  per subround (4 blocks): copy PSUM -> bf16 SBUF       (DVE)
  iou halves:     iouT = W.T@xT [+ U.T@h_tildT]         (PE)
                  i,o = sigmoid, u = tanh               (ACT)
  apply:          iu = i*u (GpSimd); cn = iu + c_agg (DVE)
                  th = tanh(cn) (ACT); hn = o*th (GpSimd)
Outputs return feature-major and permuted; the host inverts both.
"""
import os
import sys

for _p in ("/opt/trn_rl_repo",):
    if _p not in sys.path:
        sys.path.insert(0, _p)

import numpy as np
import ml_dtypes

import concourse.bass as bass
import concourse.bacc as bacc
import concourse.mybir as mybir
import concourse.tile as tile
from concourse.bass_utils import run_bass_kernel_spmd

f32 = mybir.dt.float32
bf16 = mybir.dt.bfloat16
BF = ml_dtypes.bfloat16

N_CORES = 8
BLK = 128    # destination nodes per block
G = 8        # blocks per group
GN = G * BLK # nodes per group
SUB = 4      # blocks per scatter subround
H = 256
X = 256
KH = H // 128   # 2
FO = 3 * H // 128  # 6

LAST_EXEC_TIME_NS = None
_PROGRAM_CACHE = {}


def _group_meta(ec_list, D8):
    """Per dense group: chunk table + slab offsets (shared across cores)."""
    ecs = list(ec_list)
    groups = []
    hbase = 0
    cbase = 0
    for g in range(D8 // G):
        blocks = ecs[g * G:(g + 1) * G]
        nch = sum(blocks)
        chunks = []  # (gchunk, block_local, cib, ec_of_block)
        gc = 0
        for bl, ec in enumerate(blocks):
            for c in range(ec):
                chunks.append((gc, bl, c, ec))
                gc += 1
        groups.append(dict(nch=nch, chunks=chunks, hbase=hbase, cbase=cbase))
        hbase += nch * 640
        cbase += nch * 256
    return groups, hbase, cbase


def _build_program(nblk, D8, loc, ec_list, fbias_zero):
    groups, htot, ctot = _group_meta(ec_list, D8)
    ngroups = nblk // G
    dense_groups = D8 // G
    max_h = max(g["nch"] for g in groups) * 640
    max_c = max(g["nch"] for g in groups) * 256

    nc = bacc.Bacc(None, target_bir_lowering=False, debug=False)

    hsl_d = nc.declare_dram_parameter("hslab", [128, htot], bf16, isOutput=False)
    csl_d = nc.declare_dram_parameter("cslab", [128, ctot], bf16, isOutput=False)
    xT_d = nc.declare_dram_parameter("xT", [X, loc], bf16, isOutput=False)
    ufwT_d = nc.declare_dram_parameter("ufwT", [X, H], bf16, isOutput=False)
    wiouT_d = nc.declare_dram_parameter("wiouT", [X, 3 * H], bf16, isOutput=False)
    uiouT_d = nc.declare_dram_parameter("uiouT", [H, 3 * H], bf16, isOutput=False)
    bcol_d = nc.declare_dram_parameter("bcol", [128, FO], f32, isOutput=False)
    if not fbias_zero:
        ufb_d = nc.declare_dram_parameter("ufb", [1, H], bf16, isOutput=False)
        ones_d = nc.declare_dram_parameter("ones", [1, 128], bf16, isOutput=False)

    houtT_d = nc.declare_dram_parameter("houtT", [H, loc], bf16, isOutput=True)
    coutT_d = nc.declare_dram_parameter("coutT", [H, loc], bf16, isOutput=True)

    SIG = mybir.ActivationFunctionType.Sigmoid
    TANH = mybir.ActivationFunctionType.Tanh
    MUL = mybir.AluOpType.mult
    ADD = mybir.AluOpType.add

    with tile.TileContext(nc) as tc:
        with (
            tc.tile_pool(name="const", bufs=1) as cpool,
            tc.tile_pool(name="io", bufs=2) as iop,
            tc.tile_pool(name="work", bufs=3) as wp,
            tc.tile_pool(name="grp", bufs=2) as gp,
            tc.tile_pool(name="ps_f", bufs=2, space="PSUM") as psp,
            tc.tile_pool(name="ps_iou", bufs=2, space="PSUM") as psi,
            tc.tile_pool(name="ps_acc", bufs=1, space="PSUM") as pacc,
        ):
            bcol_t = cpool.tile([128, FO], f32)
            nc.sync.dma_start(out=bcol_t[:], in_=bcol_d[:])
            ufw_t = []
            for fi in range(KH):
                t = cpool.tile([128, H], bf16, tag=f"ufw{fi}", name=f"ufw{fi}")
                nc.sync.dma_start(out=t[:], in_=ufwT_d[fi * 128:(fi + 1) * 128, :])
                ufw_t.append(t)
            wiou_t = [[None] * FO for _ in range(KH)]
            uiou_t = [[None] * FO for _ in range(KH)]
            for fi in range(KH):
                for fo in range(FO):
                    t = cpool.tile([128, 128], bf16, tag=f"wiou{fi}_{fo}",
                                   name=f"wiou{fi}_{fo}")
                    nc.sync.dma_start(
                        out=t[:], in_=wiouT_d[fi * 128:(fi + 1) * 128,
                                              fo * 128:(fo + 1) * 128])
                    wiou_t[fi][fo] = t
                    t = cpool.tile([128, 128], bf16, tag=f"uiou{fi}_{fo}",
                                   name=f"uiou{fi}_{fo}")
                    nc.sync.dma_start(
                        out=t[:], in_=uiouT_d[fi * 128:(fi + 1) * 128,
                                              fo * 128:(fo + 1) * 128])
                    uiou_t[fi][fo] = t
            if not fbias_zero:
                ones_t = cpool.tile([1, 128], bf16)
                nc.sync.dma_start(out=ones_t[:], in_=ones_d[:])
                ufb_t = cpool.tile([1, H], bf16)
                nc.sync.dma_start(out=ufb_t[:], in_=ufb_d[:])

            # apply-phase of group g is emitted after the edge phase of the
            # NEXT group, so next-group PE work is never queued behind the
            # serial iu->cn->th->hn chain on DVE/GpSimd
            def apply_phase(g, dense, sb_act, cag):
                cn3 = gp.tile([128, KH, GN], bf16, tag="cn3")
                hn3 = gp.tile([128, KH, GN], bf16, tag="hn3")
                for fi in range(KH):
                    i_sb, o_sb, u_sb = (sb_act[fi], sb_act[KH + fi],
                                        sb_act[2 * KH + fi])
                    if dense:
                        iu = wp.tile([128, GN], bf16, tag="iu")
                        nc.gpsimd.tensor_tensor(out=iu[:], in0=i_sb[:],
                                                in1=u_sb[:], op=MUL)
                        nc.vector.tensor_tensor(out=cn3[:, fi, :], in0=iu[:],
                                                in1=cag[:, fi, :], op=ADD)
                    else:
                        nc.gpsimd.tensor_tensor(out=cn3[:, fi, :], in0=i_sb[:],
                                                in1=u_sb[:], op=MUL)
                    th = wp.tile([128, GN], bf16, tag="th")
                    nc.scalar.activation(out=th[:], in_=cn3[:, fi, :],
                                         func=TANH)
                    nc.gpsimd.tensor_tensor(out=hn3[:, fi, :], in0=o_sb[:],
                                            in1=th[:], op=MUL)
                nc.gpsimd.dma_start(
                    out=coutT_d[:, g * GN:(g + 1) * GN].rearrange(
                        "(f p) c -> p f c", p=128), in_=cn3[:])
                nc.gpsimd.dma_start(
                    out=houtT_d[:, g * GN:(g + 1) * GN].rearrange(
                        "(f p) c -> p f c", p=128), in_=hn3[:])

            # interleave zero groups among dense groups so the PE never
            # sits idle through an activation-only stretch
            nz_g = ngroups - dense_groups
            keys = [((i + 0.5) / max(dense_groups, 1), i)
                    for i in range(dense_groups)]
            keys += [((j + 0.5) / max(nz_g, 1), dense_groups + j)
                     for j in range(nz_g)]
            pend = None
            cag = None
            for _, g in sorted(keys):
                dense = g < dense_groups
                xtg = gp.tile([128, KH, GN], bf16, tag="xtg")
                nc.sync.dma_start(
                    out=xtg[:],
                    in_=xT_d[:, g * GN:(g + 1) * GN].rearrange(
                        "(f p) c -> p f c", p=128))

                if dense:
                    gm = groups[g]
                    nch = gm["nch"]
                    hsl_t = iop.tile([128, max_h], bf16, tag="hsl")
                    nc.sync.dma_start(
                        out=hsl_t[:, :nch * 640],
                        in_=hsl_d[:, gm["hbase"]:gm["hbase"] + nch * 640])
                    csl_t = iop.tile([128, max_c], bf16, tag="csl")
                    nc.sync.dma_start(
                        out=csl_t[:, :nch * 256],
                        in_=csl_d[:, gm["cbase"]:gm["cbase"] + nch * 256])
                    htg = gp.tile([128, KH, GN], bf16, tag="htg")
                    cag = gp.tile([128, KH, GN], bf16, tag="cag")
                    HT0 = nch * 256  # h_childT section offset
                    S0 = nch * 512   # one-hot S section offset

                    for sub in range(2):
                        L = [ch for ch in gm["chunks"] if ch[1] // SUB == sub]
                        ht_ps = [pacc.tile([128, SUB * BLK], f32, tag=f"ht{fi}",
                                           name=f"ht_ps{fi}")
                                 for fi in range(KH)]
                        ca_ps = [pacc.tile([128, SUB * BLK], f32, tag=f"ca{fi}",
                                           name=f"ca_ps{fi}")
                                 for fi in range(KH)]
                        for p0 in range(0, len(L), 2):
                            pair = L[p0:p0 + 2]
                            w = 256 * len(pair)
                            gc0 = pair[0][0]
                            f_ps = psp.tile([128, 512], f32, tag="f")
                            for q, (gc, bl, cib, ec) in enumerate(pair):
                                for fi in range(KH):
                                    nc.tensor.matmul(
                                        out=f_ps[:, q * 256:(q + 1) * 256],
                                        lhsT=hsl_t[:, HT0 + gc * 256 + fi * 128:
                                                   HT0 + gc * 256 + fi * 128 + 128],
                                        rhs=ufw_t[fi][:],
                                        start=(fi == 0),
                                        stop=(fi == KH - 1 and fbias_zero))
                                if not fbias_zero:
                                    nc.tensor.matmul(
                                        out=f_ps[:, q * 256:(q + 1) * 256],
                                        lhsT=ones_t[:], rhs=ufb_t[:],
                                        start=False, stop=True)
                            f_sb = wp.tile([128, 512], bf16, tag="fsb")
                            nc.scalar.activation(out=f_sb[:, :w],
                                                 in_=f_ps[:, :w], func=SIG)
                            fc = wp.tile([128, 512], bf16, tag="fc")
                            nc.vector.tensor_tensor(
                                out=fc[:, :w], in0=f_sb[:, :w],
                                in1=csl_t[:, gc0 * 256:gc0 * 256 + w], op=MUL)
                            for q, (gc, bl, cib, ec) in enumerate(pair):
                                s_w = wp.tile([128, BLK], bf16, tag="S")
                                nc.vector.tensor_copy(
                                    out=s_w[:],
                                    in_=hsl_t[:, S0 + gc * 128:
                                              S0 + gc * 128 + 128])
                                s_t = s_w[:]
                                bl4 = bl % SUB
                                first = (cib == 0)
                                last = (cib == ec - 1)
                                for fi in range(KH):
                                    nc.tensor.matmul(
                                        out=ht_ps[fi][:, bl4 * BLK:(bl4 + 1) * BLK],
                                        lhsT=hsl_t[:, gc * 256 + fi * 128:
                                                   gc * 256 + fi * 128 + 128],
                                        rhs=s_t, start=first, stop=last)
                                    nc.tensor.matmul(
                                        out=ca_ps[fi][:, bl4 * BLK:(bl4 + 1) * BLK],
                                        lhsT=fc[:, q * 256 + fi * 128:
                                                q * 256 + fi * 128 + 128],
                                        rhs=s_t, start=first, stop=last)
                        SW = SUB * BLK
                        for fi in range(KH):
                            nc.vector.tensor_copy(
                                out=htg[:, fi, sub * SW:(sub + 1) * SW],
                                in_=ht_ps[fi][:])
                            nc.vector.tensor_copy(
                                out=cag[:, fi, sub * SW:(sub + 1) * SW],
                                in_=ca_ps[fi][:])

                if pend is not None:
                    apply_phase(*pend)
                    pend = None

                # ---- iou for the whole group, in 512-node halves ----
                sb_act = [wp.tile([128, GN], bf16, tag=f"act{fo}",
                                  name=f"act{fo}") for fo in range(FO)]
                for fo in range(FO):
                    for hf in range(2):
                        iou_ps = psi.tile([128, 512], f32, tag="iou")
                        for fi in range(KH):
                            nc.tensor.matmul(
                                out=iou_ps[:], lhsT=wiou_t[fi][fo][:],
                                rhs=xtg[:, fi, hf * 512:(hf + 1) * 512],
                                start=(fi == 0),
                                stop=(not dense and fi == KH - 1))
                        if dense:
                            for fi in range(KH):
                                nc.tensor.matmul(
                                    out=iou_ps[:], lhsT=uiou_t[fi][fo][:],
                                    rhs=htg[:, fi, hf * 512:(hf + 1) * 512],
                                    start=False, stop=(fi == KH - 1))
                        nc.scalar.activation(
                            out=sb_act[fo][:, hf * 512:(hf + 1) * 512],
                            in_=iou_ps[:],
                            func=(SIG if fo < 2 * KH else TANH),
                            bias=bcol_t[:, fo:fo + 1])

                pend = (g, dense, sb_act, cag if dense else None)
            if pend is not None:
                apply_phase(*pend)

    nc.compile()
    return nc


def _pack_blocks(deg_nz, D8, n2):
    """Pack nonzero-degree nodes into D8 blocks of BLK node slots.
    Blocks [0, n2) target 256 edges (2 chunks), [n2, D8) target 128
    (1 chunk). Heavy nodes (deg>=2) are snake-dealt across the first n2
    blocks, topped off with degree-1 nodes; remaining degree-1 nodes are
    dealt evenly over the 1-chunk blocks. Overflow (rare) lands in the
    emptiest block — ec is recomputed from actual counts so it stays
    correct, just costs an extra chunk."""
    nnz = deg_nz.shape[0]
    assign = np.full(nnz, -1, np.int64)
    heavy = np.flatnonzero(deg_nz >= 2)
    ones = np.flatnonzero(deg_nz == 1)
    o = np.argsort(-deg_nz[heavy], kind="stable")
    heavy = heavy[o]
    caps = np.concatenate([np.full(n2, 2 * BLK), np.full(D8 - n2, BLK)])
    rem_e = caps.astype(np.int64).copy()
    rem_n = np.full(D8, BLK, np.int64)
    nb = max(n2, 1)
    for i in range(heavy.shape[0]):
        r, j = divmod(i, nb)
        if r % 2:
            j = nb - 1 - j
        j = min(j, D8 - 1)
        assign[heavy[i]] = j
        rem_e[j] -= deg_nz[heavy[i]]
        rem_n[j] -= 1
    # top off 2-chunk blocks with degree-1 nodes
    p = 0
    for j in range(n2):
        t = int(max(0, min(rem_e[j], rem_n[j])))
        t = min(t, ones.shape[0] - p)
        if t > 0:
            assign[ones[p:p + t]] = j
            rem_e[j] -= t
            rem_n[j] -= t
            p += t
    # deal remaining degree-1 nodes evenly over 1-chunk blocks
    rest = ones[p:]
    nb1 = D8 - n2
    if rest.size and nb1 > 0:
        take = np.minimum(
            np.full(nb1, -(-rest.size // nb1), np.int64), BLK)
        take = np.minimum(take, rem_n[n2:])
        csum = np.cumsum(take)
        total = min(int(csum[-1]), rest.size)
        blocks = np.repeat(np.arange(n2, D8), take)[:total]
        assign[rest[:total]] = blocks
        np.subtract.at(rem_e, blocks, 1)
        np.subtract.at(rem_n, blocks, 1)
        rest = rest[total:]
    # overflow: emptiest-by-edges block with free slots
    for nd in rest:
        j = int(np.argmax(np.where(rem_n > 0, rem_e, -(1 << 60))))
        assign[nd] = j
        rem_e[j] -= 1
        rem_n[j] -= 1
    return assign


def _prep_inputs(x, h, c, W_iou, U_iou, b_iou, U_f_w, U_f_b,
                 edge_src, edge_dst):
    n = x.shape[0]
    assert n % N_CORES == 0
    npc = n // N_CORES
    nblk = -(-npc // GN) * G  # groups of G blocks covering npc nodes
    loc = nblk * BLK

    x = np.asarray(x, np.float32)
    h = np.asarray(h, np.float32)
    c = np.asarray(c, np.float32)
    edge_src = np.asarray(edge_src, np.int64)
    edge_dst = np.asarray(edge_dst, np.int64)

    owner = edge_dst // npc
    per_core = []
    for k in range(N_CORES):
        m = owner == k
        ldst = edge_dst[m] - k * npc
        src = edge_src[m]
        deg = np.bincount(ldst, minlength=npc)
        per_core.append((src, ldst, deg))

    nd_max = max(int((deg > 0).sum()) for _, _, deg in per_core)
    D8 = min(G * (-(-(-(-nd_max // BLK)) // G)), nblk)
    # minimal feasible count of 2-chunk blocks (see _pack_blocks)
    n2 = 1
    for _, _, deg in per_core:
        dn = deg[deg > 0]
        nH = int((dn >= 2).sum())
        EH = int(dn[dn >= 2].sum())
        nd1 = int((dn == 1).sum())
        b = max(1, -(-EH // 250))
        while b < D8:
            hj = -(-EH // b)
            nhj = -(-nH // b)
            tj = max(0, min(2 * BLK - hj, BLK - nhj))
            if nd1 - b * tj <= (D8 - b) * BLK and hj <= 2 * BLK - 6:
                break
            b += 1
        n2 = max(n2, b)

    # per-core packing of nonzero-degree nodes into dense blocks
    cores = []
    ec_arr = np.zeros((N_CORES, D8), np.int64)
    for k in range(N_CORES):
        src, ldst, deg = per_core[k]
        nz = np.flatnonzero(deg > 0)
        assign_nz = _pack_blocks(deg[nz], D8, n2)
        blk_of = np.full(npc, -1, np.int64)
        blk_of[nz] = assign_nz
        # fill remaining slots (dense leftovers first, then zero blocks)
        used = np.bincount(assign_nz, minlength=nblk)
        free_slots = BLK - used
        zeros_ = np.flatnonzero(deg == 0)
        fill_blocks = np.repeat(np.arange(nblk), free_slots)
        blk_of[zeros_] = fill_blocks[:zeros_.size]
        # position within block
        order = np.argsort(blk_of, kind="stable")
        cnt_all = np.bincount(blk_of, minlength=nblk)
        starts = np.concatenate([[0], np.cumsum(cnt_all)[:-1]])
        pos = np.empty(npc, np.int64)
        pos[order] = np.arange(npc) - np.repeat(starts, cnt_all)
        col_of = blk_of * BLK + pos
        # edge counts per dense block
        ecnt = np.bincount(blk_of[ldst], minlength=D8)[:D8]
        ec_arr[k] = -(-np.maximum(ecnt, 1) // 128)
        cores.append((src, ldst, col_of, blk_of))
    ec_list = tuple(int(v) for v in ec_arr.max(axis=0))
    ecs = np.asarray(ec_list)
    ecb = np.concatenate([[0], np.cumsum(ecs)])
    tot_ch = int(ecb[-1])
    tot_e = tot_ch * 128

    groups, htot, ctot = _group_meta(ec_list, D8)

    ufwT = np.ascontiguousarray(np.asarray(U_f_w, np.float32).T).astype(BF)
    wiouT = np.ascontiguousarray(np.asarray(W_iou, np.float32).T).astype(BF)
    uiouT = np.ascontiguousarray(np.asarray(U_iou, np.float32).T).astype(BF)
    b_iou_f = np.asarray(b_iou, np.float32).reshape(-1)
    bcol = np.ascontiguousarray(b_iou_f.reshape(FO, 128).T)
    U_f_b_f = np.asarray(U_f_b, np.float32).reshape(-1)
    fbias_zero = not U_f_b_f.any()

    in_maps = []
    col_maps = []
    for k in range(N_CORES):
        src, ldst, col_of, blk_of = cores[k]
        eblk = blk_of[ldst]
        eorder = np.argsort(eblk, kind="stable")
        cnt = np.bincount(eblk, minlength=D8)[:D8]
        start = np.zeros(D8, np.int64)
        np.cumsum(cnt[:-1], out=start[1:])
        slot_in_blk = np.arange(eblk.size) - start[eblk[eorder]]
        flat_pos = ecb[eblk[eorder]] * 128 + slot_in_blk
        hch = np.zeros((tot_e, H), np.float32)
        cch = np.zeros((tot_e, H), np.float32)
        hch[flat_pos] = h[src[eorder]]
        cch[flat_pos] = c[src[eorder]]
        flat_dst = np.full(tot_e, -1.0, np.float32)
        flat_dst[flat_pos] = (col_of[ldst[eorder]] % BLK).astype(np.float32)

        hslab = np.empty((128, htot), BF)
        cslab = np.empty((128, ctot), BF)
        # one-hot S per chunk: S[e, d] = (dst_rel[e] == d), padded rows 0
        onehot = (flat_dst[:, None] ==
                  np.arange(BLK, dtype=np.float32)[None, :]).astype(BF)
        for gm, g in zip(groups, range(len(groups))):
            nch = gm["nch"]
            e0 = int(ecb[g * G]) * 128
            hb = hch[e0:e0 + nch * 128]          # [nch*128, H]
            # edge-major: [p(edge), chunk, feat]
            p1 = hb.reshape(nch, 128, H).transpose(1, 0, 2).reshape(128, nch * H)
            # feature-major per chunk: [p(feat), chunk, fi, e]
            p2 = (hb.reshape(nch, 128, KH, 128)   # [ch, e, fi, fp]
                  .transpose(3, 0, 2, 1)          # [fp, ch, fi, e]
                  .reshape(128, nch * 256))
            hslab[:, gm["hbase"]:gm["hbase"] + nch * 256] = p1.astype(BF)
            hslab[:, gm["hbase"] + nch * 256:
                  gm["hbase"] + nch * 512] = p2.astype(BF)
            hslab[:, gm["hbase"] + nch * 512:gm["hbase"] + nch * 640] = \
                (onehot[e0:e0 + nch * 128]        # [nch*128, BLK]
                 .reshape(nch, 128, BLK).transpose(1, 0, 2)
                 .reshape(128, nch * BLK))
            cb = cch[e0:e0 + nch * 128]
            q1 = cb.reshape(nch, 128, H).transpose(1, 0, 2).reshape(128, nch * H)
            cslab[:, gm["cbase"]:gm["cbase"] + nch * 256] = q1.astype(BF)

        xT = np.zeros((X, loc), np.float32)
        xT[:, col_of] = x[k * npc:(k + 1) * npc].T
        im = {
            "hslab": hslab, "cslab": cslab, "xT": xT.astype(BF),
            "ufwT": ufwT, "wiouT": wiouT, "uiouT": uiouT,
            "bcol": bcol,
        }
        if not fbias_zero:
            im["ufb"] = U_f_b_f.reshape(1, H).astype(BF)
            im["ones"] = np.ones((1, 128), BF)
        in_maps.append(im)
        col_maps.append(col_of)

    meta = dict(n=n, npc=npc, nblk=nblk, D8=D8, loc=loc, ec_list=ec_list,
                fbias_zero=fbias_zero, col_maps=col_maps)
    return in_maps, meta


def kernel(x, h, c, W_iou, U_iou, b_iou, U_f_w, U_f_b, edge_src, edge_dst,
           _trace=False):
    global LAST_EXEC_TIME_NS
    in_maps, meta = _prep_inputs(x, h, c, W_iou, U_iou, b_iou, U_f_w, U_f_b,
                                 edge_src, edge_dst)
    key = (meta["nblk"], meta["D8"], meta["loc"], meta["ec_list"],
           meta["fbias_zero"])
    nc = _PROGRAM_CACHE.get(key)
    if nc is None:
        nc = _build_program(meta["nblk"], meta["D8"], meta["loc"],
                            meta["ec_list"], meta["fbias_zero"])
        _PROGRAM_CACHE[key] = nc
    if not _trace:
        os.environ.setdefault("BASS_NEVER_TRACE", "1")
    res = run_bass_kernel_spmd(nc, in_maps, list(range(N_CORES)),
                               trace=_trace, trace_cores=[0] if _trace else None)
    if _trace:
        LAST_EXEC_TIME_NS = res.exec_time_ns

    n = meta["n"]
    npc = meta["npc"]
    h_new = np.empty((n, H), np.float32)
    c_new = np.empty((n, H), np.float32)
    for k in range(N_CORES):
        cols = meta["col_maps"][k]
        h_new[k * npc:(k + 1) * npc] = \
            np.asarray(res.results[k]["houtT"], BF)[:, cols].T.astype(np.float32)
        c_new[k * npc:(k + 1) * npc] = \
            np.asarray(res.results[k]["coutT"], BF)[:, cols].T.astype(np.float32)
    return h_new, c_new


# revision 29
# speedup vs baseline: 1.3300x; 1.0423x over previous
"""ChildSum TreeLSTM cell for 8 Trainium2 NeuronCores — self-contained kernel.

Sharding: nodes and edges partitioned by edge_dst owner across 8 cores
(25000 nodes each). Within a core, nodes are permuted into 200 blocks of
128 destination nodes. Nodes with zero in-degree (~37%, Poisson degree)
are segregated into trailing "zero" blocks whose groups skip the U-matmul
and aggregation entirely (iou = W x only, c_new = i*u). Dense blocks are
bin-packed so block edge counts land near 128/256 (1-2 chunks of 128
edges). The host stages the halo — h[src]/c[src] rows per edge in block
order, h[src] both edge-major and feature-major — in bf16, so the device
kernel is streaming DMA + bf16 matmuls (PSUM accumulates in f32).

Device pipeline per group of 8 blocks (1024 destination nodes):
  per chunk pair: f = sigmoid(h_childT.T @ U_f)        (PE + ACT)
                  fc = f * c_child                      (DVE)
                  h_tildT += h_child.T @ S              (PE, PSUM)
                  c_aggT  += fc.T @ S                   (PE, PSUM)
  per subround (4 blocks): copy PSUM -> bf16 SBUF       (DVE)
  iou halves:     iouT = W.T@xT [+ U.T@h_tildT]         (PE)
                  i,o = sigmoid, u = tanh               (ACT)
  apply:          iu = i*u (GpSimd); cn = iu + c_agg (DVE)
                  th = tanh(cn) (ACT); hn = o*th (GpSimd)
Outputs return feature-major and permuted; the host inverts both.
"""
import os
import sys

for _p in ("/opt/trn_rl_repo",):
    if _p not in sys.path:
        sys.path.insert(0, _p)

import numpy as np
import ml_dtypes

import concourse.bass as bass
import concourse.bacc as bacc
import concourse.mybir as mybir
import concourse.tile as tile
from concourse.bass_utils import run_bass_kernel_spmd

f32 = mybir.dt.float32
bf16 = mybir.dt.bfloat16
BF = ml_dtypes.bfloat16

N_CORES = 8
BLK = 128    # destination nodes per block
G = 8        # blocks per group
GN = G * BLK # nodes per group
SUB = 4      # blocks per scatter subround
H = 256
X = 256
KH = H // 128   # 2
FO = 3 * H // 128  # 6

LAST_EXEC_TIME_NS = None
_PROGRAM_CACHE = {}


def _group_meta(ec_list, D8):
    """Per dense group: chunk table + slab offsets (shared across cores)."""
    ecs = list(ec_list)
    groups = []
    hbase = 0
    cbase = 0
    sbase = 0
    for g in range(D8 // G):
        blocks = ecs[g * G:(g + 1) * G]
        nch = sum(blocks)
        chunks = []  # (gchunk, block_local, cib, ec_of_block)
        gc = 0
        for bl, ec in enumerate(blocks):
            for c in range(ec):
                chunks.append((gc, bl, c, ec))
                gc += 1
        groups.append(dict(nch=nch, chunks=chunks, hbase=hbase, cbase=cbase,
                           sbase=sbase))
        hbase += nch * 512
        cbase += nch * 256
        sbase += nch * 128
    return groups, hbase, cbase, sbase


def _build_program(nblk, D8, loc, ec_list, fbias_zero):
    groups, htot, ctot, stot = _group_meta(ec_list, D8)
    ngroups = nblk // G
    dense_groups = D8 // G
    max_h = max(g["nch"] for g in groups) * 512
    max_c = max(g["nch"] for g in groups) * 256
    max_s = max(g["nch"] for g in groups) * 128

    nc = bacc.Bacc(None, target_bir_lowering=False, debug=False)

    hsl_d = nc.declare_dram_parameter("hslab", [128, htot], bf16, isOutput=False)
    csl_d = nc.declare_dram_parameter("cslab", [128, ctot], bf16, isOutput=False)
    ssl_d = nc.declare_dram_parameter("sslab", [128, stot], bf16, isOutput=False)
    xT_d = nc.declare_dram_parameter("xT", [X, loc], bf16, isOutput=False)
    ufwT_d = nc.declare_dram_parameter("ufwT", [X, H], bf16, isOutput=False)
    wiouT_d = nc.declare_dram_parameter("wiouT", [X, 3 * H], bf16, isOutput=False)
    uiouT_d = nc.declare_dram_parameter("uiouT", [H, 3 * H], bf16, isOutput=False)
    bcol_d = nc.declare_dram_parameter("bcol", [128, FO], f32, isOutput=False)
    if not fbias_zero:
        ufb_d = nc.declare_dram_parameter("ufb", [1, H], bf16, isOutput=False)
        ones_d = nc.declare_dram_parameter("ones", [1, 128], bf16, isOutput=False)

    houtT_d = nc.declare_dram_parameter("houtT", [H, loc], bf16, isOutput=True)
    coutT_d = nc.declare_dram_parameter("coutT", [H, loc], bf16, isOutput=True)

    SIG = mybir.ActivationFunctionType.Sigmoid
    TANH = mybir.ActivationFunctionType.Tanh
    MUL = mybir.AluOpType.mult
    ADD = mybir.AluOpType.add

    with tile.TileContext(nc) as tc:
        with (
            tc.tile_pool(name="const", bufs=1) as cpool,
            tc.tile_pool(name="io", bufs=2) as iop,
            tc.tile_pool(name="work", bufs=3) as wp,
            tc.tile_pool(name="grp", bufs=2) as gp,
            tc.tile_pool(name="ps_f", bufs=2, space="PSUM") as psp,
            tc.tile_pool(name="ps_iou", bufs=2, space="PSUM") as psi,
            tc.tile_pool(name="ps_acc", bufs=1, space="PSUM") as pacc,
        ):
            bcol_t = cpool.tile([128, FO], f32)
            nc.sync.dma_start(out=bcol_t[:], in_=bcol_d[:])
            ufw_t = []
            for fi in range(KH):
                t = cpool.tile([128, H], bf16, tag=f"ufw{fi}", name=f"ufw{fi}")
                nc.sync.dma_start(out=t[:], in_=ufwT_d[fi * 128:(fi + 1) * 128, :])
                ufw_t.append(t)
            wiou_t = [[None] * FO for _ in range(KH)]
            uiou_t = [[None] * FO for _ in range(KH)]
            for fi in range(KH):
                for fo in range(FO):
                    t = cpool.tile([128, 128], bf16, tag=f"wiou{fi}_{fo}",
                                   name=f"wiou{fi}_{fo}")
                    nc.sync.dma_start(
                        out=t[:], in_=wiouT_d[fi * 128:(fi + 1) * 128,
                                              fo * 128:(fo + 1) * 128])
                    wiou_t[fi][fo] = t
                    t = cpool.tile([128, 128], bf16, tag=f"uiou{fi}_{fo}",
                                   name=f"uiou{fi}_{fo}")
                    nc.sync.dma_start(
                        out=t[:], in_=uiouT_d[fi * 128:(fi + 1) * 128,
                                              fo * 128:(fo + 1) * 128])
                    uiou_t[fi][fo] = t
            if not fbias_zero:
                ones_t = cpool.tile([1, 128], bf16)
                nc.sync.dma_start(out=ones_t[:], in_=ones_d[:])
                ufb_t = cpool.tile([1, H], bf16)
                nc.sync.dma_start(out=ufb_t[:], in_=ufb_d[:])

            # apply-phase of group g is emitted after the edge phase of the
            # NEXT group, so next-group PE work is never queued behind the
            # serial iu->cn->th->hn chain on DVE/GpSimd
            def apply_phase(g, dense, sb_act, cag):
                cn3 = gp.tile([128, KH, GN], bf16, tag="cn3")
                hn3 = gp.tile([128, KH, GN], bf16, tag="hn3")
                for fi in range(KH):
                    i_sb, o_sb, u_sb = (sb_act[fi], sb_act[KH + fi],
                                        sb_act[2 * KH + fi])
                    if dense:
                        iu = wp.tile([128, GN], bf16, tag="iu")
                        nc.gpsimd.tensor_tensor(out=iu[:], in0=i_sb[:],
                                                in1=u_sb[:], op=MUL)
                        nc.vector.tensor_tensor(out=cn3[:, fi, :], in0=iu[:],
                                                in1=cag[:, fi, :], op=ADD)
                    else:
                        nc.gpsimd.tensor_tensor(out=cn3[:, fi, :], in0=i_sb[:],
                                                in1=u_sb[:], op=MUL)
                    th = wp.tile([128, GN], bf16, tag="th")
                    nc.scalar.activation(out=th[:], in_=cn3[:, fi, :],
                                         func=TANH)
                    nc.gpsimd.tensor_tensor(out=hn3[:, fi, :], in0=o_sb[:],
                                            in1=th[:], op=MUL)
                nc.gpsimd.dma_start(
                    out=coutT_d[:, g * GN:(g + 1) * GN].rearrange(
                        "(f p) c -> p f c", p=128), in_=cn3[:])
                nc.gpsimd.dma_start(
                    out=houtT_d[:, g * GN:(g + 1) * GN].rearrange(
                        "(f p) c -> p f c", p=128), in_=hn3[:])

            # interleave zero groups among dense groups so the PE never
            # sits idle through an activation-only stretch
            nz_g = ngroups - dense_groups
            keys = [((i + 0.5) / max(dense_groups, 1), i)
                    for i in range(dense_groups)]
            keys += [((j + 0.5) / max(nz_g, 1), dense_groups + j)
                     for j in range(nz_g)]
            pend = None
            cag = None
            for _, g in sorted(keys):
                dense = g < dense_groups
                xtg = gp.tile([128, KH, GN], bf16, tag="xtg")
                nc.sync.dma_start(
                    out=xtg[:],
                    in_=xT_d[:, g * GN:(g + 1) * GN].rearrange(
                        "(f p) c -> p f c", p=128))

                if dense:
                    gm = groups[g]
                    nch = gm["nch"]
                    hsl_t = iop.tile([128, max_h], bf16, tag="hsl")
                    nc.sync.dma_start(
                        out=hsl_t[:, :nch * 512],
                        in_=hsl_d[:, gm["hbase"]:gm["hbase"] + nch * 512])
                    ssl_t = iop.tile([128, max_s], bf16, tag="ssl")
                    nc.sync.dma_start(
                        out=ssl_t[:, :nch * 128],
                        in_=ssl_d[:, gm["sbase"]:gm["sbase"] + nch * 128])
                    csl_t = iop.tile([128, max_c], bf16, tag="csl")
                    nc.sync.dma_start(
                        out=csl_t[:, :nch * 256],
                        in_=csl_d[:, gm["cbase"]:gm["cbase"] + nch * 256])
                    htg = gp.tile([128, KH, GN], bf16, tag="htg")
                    cag = gp.tile([128, KH, GN], bf16, tag="cag")
                    HT0 = nch * 256  # h_childT section offset

                    for sub in range(2):
                        L = [ch for ch in gm["chunks"] if ch[1] // SUB == sub]
                        ht_ps = [pacc.tile([128, SUB * BLK], f32, tag=f"ht{fi}",
                                           name=f"ht_ps{fi}")
                                 for fi in range(KH)]
                        ca_ps = [pacc.tile([128, SUB * BLK], f32, tag=f"ca{fi}",
                                           name=f"ca_ps{fi}")
                                 for fi in range(KH)]
                        for p0 in range(0, len(L), 2):
                            pair = L[p0:p0 + 2]
                            w = 256 * len(pair)
                            gc0 = pair[0][0]
                            f_ps = psp.tile([128, 512], f32, tag="f")
                            for q, (gc, bl, cib, ec) in enumerate(pair):
                                for fi in range(KH):
                                    nc.tensor.matmul(
                                        out=f_ps[:, q * 256:(q + 1) * 256],
                                        lhsT=hsl_t[:, HT0 + gc * 256 + fi * 128:
                                                   HT0 + gc * 256 + fi * 128 + 128],
                                        rhs=ufw_t[fi][:],
                                        start=(fi == 0),
                                        stop=(fi == KH - 1 and fbias_zero))
                                if not fbias_zero:
                                    nc.tensor.matmul(
                                        out=f_ps[:, q * 256:(q + 1) * 256],
                                        lhsT=ones_t[:], rhs=ufb_t[:],
                                        start=False, stop=True)
                            f_sb = wp.tile([128, 512], bf16, tag="fsb")
                            nc.scalar.activation(out=f_sb[:, :w],
                                                 in_=f_ps[:, :w], func=SIG)
                            fc = wp.tile([128, 512], bf16, tag="fc")
                            nc.vector.tensor_tensor(
                                out=fc[:, :w], in0=f_sb[:, :w],
                                in1=csl_t[:, gc0 * 256:gc0 * 256 + w], op=MUL)
                            for q, (gc, bl, cib, ec) in enumerate(pair):
                                s_t = ssl_t[:, gc * 128:gc * 128 + 128]
                                bl4 = bl % SUB
                                first = (cib == 0)
                                last = (cib == ec - 1)
                                for fi in range(KH):
                                    nc.tensor.matmul(
                                        out=ht_ps[fi][:, bl4 * BLK:(bl4 + 1) * BLK],
                                        lhsT=hsl_t[:, gc * 256 + fi * 128:
                                                   gc * 256 + fi * 128 + 128],
                                        rhs=s_t, start=first, stop=last)
                                    nc.tensor.matmul(
                                        out=ca_ps[fi][:, bl4 * BLK:(bl4 + 1) * BLK],
                                        lhsT=fc[:, q * 256 + fi * 128:
                                                q * 256 + fi * 128 + 128],
                                        rhs=s_t, start=first, stop=last)
                        SW = SUB * BLK
                        for fi in range(KH):
                            nc.vector.tensor_copy(
                                out=htg[:, fi, sub * SW:(sub + 1) * SW],
                                in_=ht_ps[fi][:])
                            nc.vector.tensor_copy(
                                out=cag[:, fi, sub * SW:(sub + 1) * SW],
                                in_=ca_ps[fi][:])

                if pend is not None:
                    apply_phase(*pend)
                    pend = None

                # ---- iou for the whole group, in 512-node halves ----
                sb_act = [wp.tile([128, GN], bf16, tag=f"act{fo}",
                                  name=f"act{fo}") for fo in range(FO)]
                for fo in range(FO):
                    for hf in range(2):
                        iou_ps = psi.tile([128, 512], f32, tag="iou")
                        for fi in range(KH):
                            nc.tensor.matmul(
                                out=iou_ps[:], lhsT=wiou_t[fi][fo][:],
                                rhs=xtg[:, fi, hf * 512:(hf + 1) * 512],
                                start=(fi == 0),
                                stop=(not dense and fi == KH - 1))
                        if dense:
                            for fi in range(KH):
                                nc.tensor.matmul(
                                    out=iou_ps[:], lhsT=uiou_t[fi][fo][:],
                                    rhs=htg[:, fi, hf * 512:(hf + 1) * 512],
                                    start=False, stop=(fi == KH - 1))
                        nc.scalar.activation(
                            out=sb_act[fo][:, hf * 512:(hf + 1) * 512],
                            in_=iou_ps[:],
                            func=(SIG if fo < 2 * KH else TANH),
                            bias=bcol_t[:, fo:fo + 1])

                pend = (g, dense, sb_act, cag if dense else None)
            if pend is not None:
                apply_phase(*pend)

    nc.compile()
    return nc


def _pack_blocks(deg_nz, D8, n2):
    """Pack nonzero-degree nodes into D8 blocks of BLK node slots.
    Blocks [0, n2) target 256 edges (2 chunks), [n2, D8) target 128
    (1 chunk). Heavy nodes (deg>=2) are snake-dealt across the first n2
    blocks, topped off with degree-1 nodes; remaining degree-1 nodes are
    dealt evenly over the 1-chunk blocks. Overflow (rare) lands in the
    emptiest block — ec is recomputed from actual counts so it stays
    correct, just costs an extra chunk."""
    nnz = deg_nz.shape[0]
    assign = np.full(nnz, -1, np.int64)
    heavy = np.flatnonzero(deg_nz >= 2)
    ones = np.flatnonzero(deg_nz == 1)
    o = np.argsort(-deg_nz[heavy], kind="stable")
    heavy = heavy[o]
    caps = np.concatenate([np.full(n2, 2 * BLK), np.full(D8 - n2, BLK)])
    rem_e = caps.astype(np.int64).copy()
    rem_n = np.full(D8, BLK, np.int64)
    nb = max(n2, 1)
    for i in range(heavy.shape[0]):
        r, j = divmod(i, nb)
        if r % 2:
            j = nb - 1 - j
        j = min(j, D8 - 1)
        assign[heavy[i]] = j
        rem_e[j] -= deg_nz[heavy[i]]
        rem_n[j] -= 1
    # top off 2-chunk blocks with degree-1 nodes
    p = 0
    for j in range(n2):
        t = int(max(0, min(rem_e[j], rem_n[j])))
        t = min(t, ones.shape[0] - p)
        if t > 0:
            assign[ones[p:p + t]] = j
            rem_e[j] -= t
            rem_n[j] -= t
            p += t
    # deal remaining degree-1 nodes evenly over 1-chunk blocks
    rest = ones[p:]
    nb1 = D8 - n2
    if rest.size and nb1 > 0:
        take = np.minimum(
            np.full(nb1, -(-rest.size // nb1), np.int64), BLK)
        take = np.minimum(take, rem_n[n2:])
        csum = np.cumsum(take)
        total = min(int(csum[-1]), rest.size)
        blocks = np.repeat(np.arange(n2, D8), take)[:total]
        assign[rest[:total]] = blocks
        np.subtract.at(rem_e, blocks, 1)
        np.subtract.at(rem_n, blocks, 1)
        rest = rest[total:]
    # overflow: emptiest-by-edges block with free slots
    for nd in rest:
        j = int(np.argmax(np.where(rem_n > 0, rem_e, -(1 << 60))))
        assign[nd] = j
        rem_e[j] -= 1
        rem_n[j] -= 1
    return assign


def _prep_inputs(x, h, c, W_iou, U_iou, b_iou, U_f_w, U_f_b,
                 edge_src, edge_dst):
    n = x.shape[0]
    assert n % N_CORES == 0
    npc = n // N_CORES
    nblk = -(-npc // GN) * G  # groups of G blocks covering npc nodes
    loc = nblk * BLK

    x = np.asarray(x, np.float32)
    h = np.asarray(h, np.float32)
    c = np.asarray(c, np.float32)
    edge_src = np.asarray(edge_src, np.int64)
    edge_dst = np.asarray(edge_dst, np.int64)

    owner = edge_dst // npc
    per_core = []
    for k in range(N_CORES):
        m = owner == k
        ldst = edge_dst[m] - k * npc
        src = edge_src[m]
        deg = np.bincount(ldst, minlength=npc)
        per_core.append((src, ldst, deg))

    nd_max = max(int((deg > 0).sum()) for _, _, deg in per_core)
    D8 = min(G * (-(-(-(-nd_max // BLK)) // G)), nblk)
    # minimal feasible count of 2-chunk blocks (see _pack_blocks)
    n2 = 1
    for _, _, deg in per_core:
        dn = deg[deg > 0]
        nH = int((dn >= 2).sum())
        EH = int(dn[dn >= 2].sum())
        nd1 = int((dn == 1).sum())
        b = max(1, -(-EH // 250))
        while b < D8:
            hj = -(-EH // b)
            nhj = -(-nH // b)
            tj = max(0, min(2 * BLK - hj, BLK - nhj))
            if nd1 - b * tj <= (D8 - b) * BLK and hj <= 2 * BLK - 6:
                break
            b += 1
        n2 = max(n2, b)

    # per-core packing of nonzero-degree nodes into dense blocks
    cores = []
    ec_arr = np.zeros((N_CORES, D8), np.int64)
    for k in range(N_CORES):
        src, ldst, deg = per_core[k]
        nz = np.flatnonzero(deg > 0)
        assign_nz = _pack_blocks(deg[nz], D8, n2)
        blk_of = np.full(npc, -1, np.int64)
        blk_of[nz] = assign_nz
        # fill remaining slots (dense leftovers first, then zero blocks)
        used = np.bincount(assign_nz, minlength=nblk)
        free_slots = BLK - used
        zeros_ = np.flatnonzero(deg == 0)
        fill_blocks = np.repeat(np.arange(nblk), free_slots)
        blk_of[zeros_] = fill_blocks[:zeros_.size]
        # position within block
        order = np.argsort(blk_of, kind="stable")
        cnt_all = np.bincount(blk_of, minlength=nblk)
        starts = np.concatenate([[0], np.cumsum(cnt_all)[:-1]])
        pos = np.empty(npc, np.int64)
        pos[order] = np.arange(npc) - np.repeat(starts, cnt_all)
        col_of = blk_of * BLK + pos
        # edge counts per dense block
        ecnt = np.bincount(blk_of[ldst], minlength=D8)[:D8]
        ec_arr[k] = -(-np.maximum(ecnt, 1) // 128)
        cores.append((src, ldst, col_of, blk_of))
    ec_list = tuple(int(v) for v in ec_arr.max(axis=0))
    ecs = np.asarray(ec_list)
    ecb = np.concatenate([[0], np.cumsum(ecs)])
    tot_ch = int(ecb[-1])
    tot_e = tot_ch * 128

    groups, htot, ctot, stot = _group_meta(ec_list, D8)

    ufwT = np.ascontiguousarray(np.asarray(U_f_w, np.float32).T).astype(BF)
    wiouT = np.ascontiguousarray(np.asarray(W_iou, np.float32).T).astype(BF)
    uiouT = np.ascontiguousarray(np.asarray(U_iou, np.float32).T).astype(BF)
    b_iou_f = np.asarray(b_iou, np.float32).reshape(-1)
    bcol = np.ascontiguousarray(b_iou_f.reshape(FO, 128).T)
    U_f_b_f = np.asarray(U_f_b, np.float32).reshape(-1)
    fbias_zero = not U_f_b_f.any()

    in_maps = []
    col_maps = []
    for k in range(N_CORES):
        src, ldst, col_of, blk_of = cores[k]
        eblk = blk_of[ldst]
        eorder = np.argsort(eblk, kind="stable")
        cnt = np.bincount(eblk, minlength=D8)[:D8]
        start = np.zeros(D8, np.int64)
        np.cumsum(cnt[:-1], out=start[1:])
        slot_in_blk = np.arange(eblk.size) - start[eblk[eorder]]
        flat_pos = ecb[eblk[eorder]] * 128 + slot_in_blk
        hch = np.zeros((tot_e, H), np.float32)
        cch = np.zeros((tot_e, H), np.float32)
        hch[flat_pos] = h[src[eorder]]
        cch[flat_pos] = c[src[eorder]]
        flat_dst = np.full(tot_e, -1.0, np.float32)
        flat_dst[flat_pos] = (col_of[ldst[eorder]] % BLK).astype(np.float32)

        hslab = np.empty((128, htot), BF)
        cslab = np.empty((128, ctot), BF)
        sslab = np.empty((128, stot), BF)
        # one-hot S per chunk: S[e, d] = (dst_rel[e] == d), padded rows 0
        onehot = (flat_dst[:, None] ==
                  np.arange(BLK, dtype=np.float32)[None, :]).astype(BF)
        for gm, g in zip(groups, range(len(groups))):
            nch = gm["nch"]
            e0 = int(ecb[g * G]) * 128
            hb = hch[e0:e0 + nch * 128]          # [nch*128, H]
            # edge-major: [p(edge), chunk, feat]
            p1 = hb.reshape(nch, 128, H).transpose(1, 0, 2).reshape(128, nch * H)
            # feature-major per chunk: [p(feat), chunk, fi, e]
            p2 = (hb.reshape(nch, 128, KH, 128)   # [ch, e, fi, fp]
                  .transpose(3, 0, 2, 1)          # [fp, ch, fi, e]
                  .reshape(128, nch * 256))
            hslab[:, gm["hbase"]:gm["hbase"] + nch * 256] = p1.astype(BF)
            hslab[:, gm["hbase"] + nch * 256:
                  gm["hbase"] + nch * 512] = p2.astype(BF)
            sslab[:, gm["sbase"]:gm["sbase"] + nch * 128] = \
                (onehot[e0:e0 + nch * 128]        # [nch*128, BLK]
                 .reshape(nch, 128, BLK).transpose(1, 0, 2)
                 .reshape(128, nch * BLK))
            cb = cch[e0:e0 + nch * 128]
            q1 = cb.reshape(nch, 128, H).transpose(1, 0, 2).reshape(128, nch * H)
            cslab[:, gm["cbase"]:gm["cbase"] + nch * 256] = q1.astype(BF)

        xT = np.zeros((X, loc), np.float32)
        xT[:, col_of] = x[k * npc:(k + 1) * npc].T
        im = {
            "hslab": hslab, "cslab": cslab, "sslab": sslab, "xT": xT.astype(BF),
            "ufwT": ufwT, "wiouT": wiouT, "uiouT": uiouT,
            "bcol": bcol,
        }
        if not fbias_zero:
            im["ufb"] = U_f_b_f.reshape(1, H).astype(BF)
            im["ones"] = np.ones((1, 128), BF)
        in_maps.append(im)
        col_maps.append(col_of)

    meta = dict(n=n, npc=npc, nblk=nblk, D8=D8, loc=loc, ec_list=ec_list,
                fbias_zero=fbias_zero, col_maps=col_maps)
    return in_maps, meta


def kernel(x, h, c, W_iou, U_iou, b_iou, U_f_w, U_f_b, edge_src, edge_dst,
           _trace=False):
    global LAST_EXEC_TIME_NS
    in_maps, meta = _prep_inputs(x, h, c, W_iou, U_iou, b_iou, U_f_w, U_f_b,
                                 edge_src, edge_dst)
    key = (meta["nblk"], meta["D8"], meta["loc"], meta["ec_list"],
           meta["fbias_zero"])
    nc = _PROGRAM_CACHE.get(key)
    if nc is None:
        nc = _build_program(meta["nblk"], meta["D8"], meta["loc"],
                            meta["ec_list"], meta["fbias_zero"])
        _PROGRAM_CACHE[key] = nc
    if not _trace:
        os.environ.setdefault("BASS_NEVER_TRACE", "1")
    res = run_bass_kernel_spmd(nc, in_maps, list(range(N_CORES)),
                               trace=_trace, trace_cores=[0] if _trace else None)
    if _trace:
        LAST_EXEC_TIME_NS = res.exec_time_ns

    n = meta["n"]
    npc = meta["npc"]
    h_new = np.empty((n, H), np.float32)
    c_new = np.empty((n, H), np.float32)
    for k in range(N_CORES):
        cols = meta["col_maps"][k]
        h_new[k * npc:(k + 1) * npc] = \
            np.asarray(res.results[k]["houtT"], BF)[:, cols].T.astype(np.float32)
        c_new[k * npc:(k + 1) * npc] = \
            np.asarray(res.results[k]["coutT"], BF)[:, cols].T.astype(np.float32)
    return h_new, c_new
